# revision 1
# baseline (speedup 1.0000x reference)
"""Trainium2 Bass kernel for nn_NodeEdgeCrossAttention.

Strategy (dst-sharded, zero-collective):
  - Host sorts edges by destination node, assigns nodes to 8 cores with
    balanced padded-edge counts, and packs each node's edge run (padded to a
    multiple of 32) into 512-column chunks using a slot pattern shared by all
    cores (SPMD requires one program).  Each chunk holds at most 8 slots;
    slot s of chunk c gets global index c*8+s.
  - Scores fold Wq/Wk into per-node M matrices (score = M[dst] . k_raw), so
    no k-projection or q-gather is needed.  bk cancels by softmax shift
    invariance; bv folds through Wo into bo because sum(attn) == 1.
  - Per chunk: one fused kvs DMA (k | v | one-hot S), per-slot score matmuls,
    one exp, one DMA-transpose for edge-major exp values, 4 v-projection
    matmuls, one fused weighted-v multiply, and 4 segment matmuls with the
    one-hot S slot columns as weights accumulating [8 slots, 144] in PSUM
    (seg sums and softmax denominators together).  Park groups of 3 chunks
    drain to a DRAM scratch by DMA.
  - Numerics: fp16 for linear tensors, bf16 for exp-range tensors, fp32
    accumulation; validated at ~2e-3 max relative error.
"""

import numpy as np

N, E, DIM, HEADS = 10000, 640000, 128, 4
DH = DIM // HEADS
NCORES = 8
CHUNK = 512
TILE = 128
SCALE = DH ** -0.5
SP = 16              # exp staging columns per tile
PW = DIM + HEADS     # 132: per-tile rhs width (exv | exE)
GPC = 3              # chunks per PSUM park group


class Plan:
    pass


def _make_plan(dst):
    """Pack nodes into a chunk/slot layout shared across all 8 cores."""
    deg = np.bincount(dst, minlength=N)
    if deg.max() > 128:
        raise NotImplementedError(f"max degree {deg.max()} > 128 needs node splitting")
    Rn = np.maximum(np.ceil(deg / 32.0).astype(np.int64), 1) * 32

    order = np.argsort(-Rn, kind="stable")
    loads = np.zeros(NCORES, np.int64)
    core_nodes = [[] for _ in range(NCORES)]
    for n in order:
        c = int(loads.argmin())
        core_nodes[c].append(int(n))
        loads[c] += Rn[n]

    # Shared slot pattern = elementwise max over cores' (desc-sorted) R seqs.
    L = max(len(cn) for cn in core_nodes)
    pat = np.zeros(L, np.int64)
    for cn in core_nodes:
        r = Rn[np.array(cn, np.int64)]
        pat[: len(r)] = np.maximum(pat[: len(r)], r)

    slots = []           # {R, chunk, col0, pi}
    chunks = []          # {slots: [slot indices]}
    cur = {"slots": []}
    rem = CHUNK
    pi = 0
    while pi < L:
        R = int(pat[pi])
        if R <= rem:
            cur["slots"].append(len(slots))
            slots.append({"R": R, "chunk": len(chunks), "col0": CHUNK - rem, "pi": pi})
            rem -= R
            pi += 1
        else:
            if rem > 0:
                cur["slots"].append(len(slots))
                slots.append({"R": rem, "chunk": len(chunks),
                              "col0": CHUNK - rem, "pi": -1})
            chunks.append(cur)
            cur = {"slots": []}
            rem = CHUNK
    if rem > 0 and rem < CHUNK:
        cur["slots"].append(len(slots))
        slots.append({"R": rem, "chunk": len(chunks), "col0": CHUNK - rem, "pi": -1})
    if cur["slots"]:
        chunks.append(cur)

    max_ns = 0
    for ch in chunks:
        ch["ns"] = len(ch["slots"])
        max_ns = max(max_ns, ch["ns"])

    p = Plan()
    p.sl = max_ns                                    # slot positions per chunk
    p.kvw = 2 * CHUNK + 4 * p.sl
    p.deg = deg
    p.core_nodes = core_nodes
    p.slots = slots
    p.chunks = chunks
    p.nchunks = len(chunks)
    p.cols = p.nchunks * CHUNK
    p.nslot = p.nchunks * p.sl                       # sparse slot space
    p.nslot_b = ((p.nslot + TILE - 1) // TILE) * TILE    # 128-padded
    p.nsp = ((p.nslot + CHUNK - 1) // CHUNK) * CHUNK     # 512-padded
    return p


def _pack_core_inputs(plan, c, k_edges, v_edges, q_nodes, edges_of):
    """Per-core fused kvs [128, nchunks*KVW] f16, qT [128, nsp] f16, qslot."""
    import ml_dtypes
    cols = plan.cols
    edge_order = np.full(cols, -1, np.int64)
    qslot = np.full(plan.nslot, -1, np.int64)
    cn = plan.core_nodes[c]
    for ch_i, ch in enumerate(plan.chunks):
        for j, sidx in enumerate(ch["slots"]):
            s = plan.slots[sidx]
            if s["pi"] < 0 or s["pi"] >= len(cn):
                continue
            node = cn[s["pi"]]
            d = plan.deg[node]
            g0 = ch_i * CHUNK + s["col0"]
            edge_order[g0: g0 + d] = edges_of[node]
            qslot[ch_i * plan.sl + j] = node

    valid = edge_order >= 0
    idx = np.where(valid, edge_order, 0)
    kT = np.where(valid[:, None], k_edges[idx], 0.0).astype(np.float16).T
    vT = np.where(valid[:, None], v_edges[idx], 0.0).astype(np.float16).T

    # one-hot S: [128, nchunks*4*SLOTS], col (chunk, tile, slot_j)
    S = np.zeros((TILE, plan.nchunks * 4 * plan.sl), np.float32)
    for ci, ch in enumerate(plan.chunks):
        for j, sidx in enumerate(ch["slots"]):
            s = plan.slots[sidx]
            if s["pi"] < 0 or s["pi"] >= len(cn):
                continue
            d = int(plan.deg[cn[s["pi"]]])
            for t in range(4):
                lo = max(s["col0"], t * TILE)
                hi = min(s["col0"] + d, (t + 1) * TILE)
                if lo < hi:
                    S[lo - t * TILE:hi - t * TILE, (ci * 4 + t) * plan.sl + j] = 1.0
    Sbits = S.astype(ml_dtypes.bfloat16).view(np.float16)

    kvs = np.empty((TILE, plan.nchunks * plan.kvw), np.float16)
    kc = kT.reshape(TILE, plan.nchunks, CHUNK)
    vc = vT.reshape(TILE, plan.nchunks, CHUNK)
    sc = Sbits.reshape(TILE, plan.nchunks, 4 * plan.sl)
    kvw = kvs.reshape(TILE, plan.nchunks, plan.kvw)
    kvw[:, :, 0:CHUNK] = kc
    kvw[:, :, CHUNK:2 * CHUNK] = vc
    kvw[:, :, 2 * CHUNK:plan.kvw] = sc

    qvalid = qslot >= 0
    qidx = np.where(qvalid, qslot, 0)
    qT = np.zeros((DIM, plan.nsp), np.float16)
    qT[:, : plan.nslot] = np.where(qvalid[:, None], q_nodes[qidx], 0.0
                                   ).astype(np.float16).T
    return kvs, qT, qslot


# ---------------------------------------------------------------------------
# Device kernel emission
# ---------------------------------------------------------------------------

def _build_module(plan):
    import concourse.bacc as bacc
    import concourse.mybir as mybir
    import concourse.tile as tile
    from contextlib import ExitStack

    f16 = mybir.dt.float16
    bf = mybir.dt.bfloat16
    f32 = mybir.dt.float32
    NSP = plan.nsp
    NBLK = plan.nslot_b // TILE
    CW = PW              # 132 scratch row width
    SL = plan.sl
    KVW = plan.kvw

    nc = bacc.Bacc("TRN2", debug=False, num_devices=NCORES)

    kvs_d = nc.dram_tensor("kvs", [TILE, plan.nchunks * KVW], f16,
                           kind="ExternalInput")
    qT_d = nc.dram_tensor("qT", [DIM, NSP], f16, kind="ExternalInput")
    Wq_d = nc.dram_tensor("Wq", [DIM, DIM], f16, kind="ExternalInput")
    WkTs_d = nc.dram_tensor("WkTs", [DIM, DIM], f16, kind="ExternalInput")
    Wv_d = nc.dram_tensor("Wv", [DIM, DIM], f16, kind="ExternalInput")
    Wo_d = nc.dram_tensor("Wo", [DIM, DIM], f32, kind="ExternalInput")
    Hm_d = nc.dram_tensor("Hm", [DIM, HEADS], f16, kind="ExternalInput")
    ID_d = nc.dram_tensor("ID", [DIM, DIM], f32, kind="ExternalInput")
    I4_d = nc.dram_tensor("I4", [HEADS, HEADS], bf, kind="ExternalInput")
    bq_d = nc.dram_tensor("bq", [DIM, 1], f32, kind="ExternalInput")
    bo_d = nc.dram_tensor("bo", [DIM, 1], f32, kind="ExternalInput")
    accD = nc.dram_tensor("accD", [plan.nslot, CW], f32, kind="Internal")
    outT_d = nc.dram_tensor("outT", [DIM, NSP], f32, kind="ExternalOutput")

    Exp = mybir.ActivationFunctionType.Exp
    Ident = mybir.ActivationFunctionType.Identity
    mult = mybir.AluOpType.mult
    amax = mybir.AluOpType.max

    with ExitStack() as ctx:
        tc = ctx.enter_context(tile.TileContext(nc))
        cp = ctx.enter_context(tc.tile_pool(name="const", bufs=1))
        sp = ctx.enter_context(tc.tile_pool(name="persist", bufs=1))
        iop = ctx.enter_context(tc.tile_pool(name="io", bufs=4))
        xp = ctx.enter_context(tc.tile_pool(name="work", bufs=4))
        pp = ctx.enter_context(tc.tile_pool(name="ps", bufs=2, space="PSUM"))

        def dmac(tile_ap, dram_ap):
            nc.sync.dma_start(out=tile_ap, in_=dram_ap)

        Wq_sb = cp.tile([DIM, DIM], f16); dmac(Wq_sb[:], Wq_d[:, :])
        WkTs_sb = cp.tile([DIM, DIM], f16); dmac(WkTs_sb[:], WkTs_d[:, :])
        Wv_sb = cp.tile([DIM, DIM], f16); dmac(Wv_sb[:], Wv_d[:, :])
        Wo_sb = cp.tile([DIM, DIM], f32); dmac(Wo_sb[:], Wo_d[:, :])
        Hm_sb = cp.tile([DIM, HEADS], f16); dmac(Hm_sb[:], Hm_d[:, :])
        ID_sb = cp.tile([DIM, DIM], f32); dmac(ID_sb[:], ID_d[:, :])
        I4_sb = cp.tile([HEADS, HEADS], bf); dmac(I4_sb[:], I4_d[:, :])
        bq_sb = cp.tile([DIM, 1], f32); dmac(bq_sb[:], bq_d[:, :])
        bo_sb = cp.tile([DIM, 1], f32); dmac(bo_sb[:], bo_d[:, :])
        qT_sb = sp.tile([DIM, NSP], f16); dmac(qT_sb[:], qT_d[:, :])

        qp_sb = sp.tile([DIM, NSP], f16)
        M_sb = sp.tile([DIM, 4 * NSP], f16)

        # ---- Stage A: q projection + bias ----
        for b in range(NSP // CHUNK):
            sl = slice(b * CHUNK, (b + 1) * CHUNK)
            qp_ps = pp.tile([DIM, CHUNK], f32, tag="aux")
            nc.tensor.matmul(out=qp_ps[:], lhsT=Wq_sb[:], rhs=qT_sb[:, sl],
                             start=True, stop=True)
            nc.scalar.activation(out=qp_sb[:, sl], in_=qp_ps[:],
                                 func=Ident, bias=bq_sb[:, 0:1])

        # ---- Stage A: M matrices, 32 slots per group ----
        for g in range(NSP // 32):
            qsl = slice(g * 32, (g + 1) * 32)
            qpm = xp.tile([DIM, TILE], f16, tag="qpm")
            nc.vector.tensor_tensor(
                out=qpm[:].rearrange("p (w h) -> p w h", h=HEADS),
                in0=qp_sb[:, qsl].unsqueeze(-1).to_broadcast([DIM, 32, HEADS]),
                in1=Hm_sb[:, :].unsqueeze(1).to_broadcast([DIM, 32, HEADS]),
                op=mult)
            M_ps = pp.tile([DIM, TILE], f32, tag="aux")
            nc.tensor.matmul(out=M_ps[:], lhsT=WkTs_sb[:], rhs=qpm[:],
                             start=True, stop=True)
            nc.scalar.copy(out=M_sb[:, g * TILE:(g + 1) * TILE], in_=M_ps[:])

        # ---- Steady state ----
        park = None
        for ci, ch in enumerate(plan.chunks):
            kvt = iop.tile([TILE, KVW], f16, tag="kv")
            dmac(kvt[:], kvs_d[:, ci * KVW:(ci + 1) * KVW])
            kc = kvt[:, 0:CHUNK]
            vc = kvt[:, CHUNK:2 * CHUNK]
            Sc = kvt[:, 2 * CHUNK:KVW].bitcast(bf)

            score_ps = pp.tile([HEADS, CHUNK], f32, tag="score")
            for j, sidx in enumerate(ch["slots"]):
                s = plan.slots[sidx]
                g = ci * SL + j
                c0 = s["col0"]
                nc.tensor.matmul(
                    out=score_ps[0:HEADS, c0:c0 + s["R"]],
                    lhsT=M_sb[:, 4 * g:4 * g + 4],
                    rhs=kc[:, c0:c0 + s["R"]],
                    start=True, stop=True)

            ex_sb = xp.tile([HEADS, CHUNK], bf, tag="ex")
            nc.scalar.activation(out=ex_sb[:], in_=score_ps[:], func=Exp)
            exT_ps = pp.tile([TILE, 4 * HEADS], f32, tag="aux")
            for t in range(4):
                nc.tensor.matmul(
                    out=exT_ps[:, 4 * t:4 * t + 4],
                    lhsT=ex_sb[0:HEADS, t * TILE:(t + 1) * TILE],
                    rhs=I4_sb[:], start=True, stop=True)
            exE_sb = xp.tile([TILE, 4 * HEADS], bf, tag="exE")
            nc.scalar.copy(out=exE_sb[:], in_=exT_ps[:])

            vp_ps = pp.tile([TILE, CHUNK], f32, tag="vp")
            for t in range(4):
                nc.tensor.matmul(
                    out=vp_ps[:, t * TILE:(t + 1) * TILE],
                    lhsT=vc[:, t * TILE:(t + 1) * TILE],
                    rhs=Wv_sb[:], start=True, stop=True)

            exF_sb = xp.tile([TILE, 4 * PW], bf, tag="exF")
            exF_t = exF_sb[:].rearrange("p (t c) -> p t c", t=4)
            nc.vector.tensor_tensor(
                out=exF_t[:, :, 0:DIM].rearrange("p t (h d) -> p t h d", h=HEADS),
                in0=vp_ps[:].rearrange("p (t h d) -> p t h d", t=4, h=HEADS),
                in1=exE_sb[:].rearrange("p (t h) -> p t h", t=4)
                    .unsqueeze(-1).to_broadcast([TILE, 4, HEADS, DH]),
                op=mult)
            nc.scalar.copy(
                out=exF_t[:, :, DIM:PW],
                in_=exE_sb[:].rearrange("p (t h) -> p t h", t=4))

            gi = ci % GPC
            if gi == 0:
                park = pp.tile([SL, GPC * CW], f32, tag="park")
            for t in range(4):
                nc.tensor.matmul(
                    out=park[:, gi * CW:(gi + 1) * CW],
                    lhsT=Sc[:, t * SL:(t + 1) * SL],
                    rhs=exF_t[:, t, :],
                    start=(t == 0), stop=(t == 3))
            if gi == GPC - 1 or ci == plan.nchunks - 1:
                g0 = (ci // GPC) * GPC
                used = ci - g0 + 1
                stage = xp.tile([SL, GPC * CW], f32, tag="stage")
                nc.vector.tensor_copy(out=stage[:, 0:used * CW],
                                      in_=park[:, 0:used * CW])
                nc.scalar.dma_start(
                    out=accD[g0 * SL:(ci + 1) * SL, :]
                        .rearrange("(c j) w -> j c w", j=SL),
                    in_=stage[:, 0:used * CW]
                        .rearrange("j (c w) -> j c w", w=CW))

        # ---- Final: read scratch back aligned, normalize, project ----
        accR = sp.tile([TILE, NBLK * CW], f32)
        nc.gpsimd.memset(accR[:], 0.0)
        full = plan.nslot // TILE          # whole 128-row blocks
        if full:
            dmac(accR[:, 0:full * CW]
                 .rearrange("p (b w) -> p b w", w=CW),
                 accD[0:full * TILE, :].rearrange("(b p) w -> p b w", p=TILE))
        tail = plan.nslot - full * TILE
        if tail:
            dmac(accR[0:tail, full * CW:(full + 1) * CW],
                 accD[full * TILE:plan.nslot, :])

        rden_sb = sp.tile([TILE, NBLK * HEADS], f32)
        nc.vector.tensor_scalar(
            out=rden_sb[:].rearrange("p (b h) -> p b h", h=HEADS),
            in0=accR[:].rearrange("p (b w) -> p b w", w=CW)[:, :, DIM:DIM + HEADS],
            scalar1=1e-30, scalar2=None, op0=amax)
        nc.vector.reciprocal(out=rden_sb[:], in_=rden_sb[:])
        agg_sb = sp.tile([TILE, NBLK * DIM], f32)
        nc.vector.tensor_tensor(
            out=agg_sb[:].rearrange("p (b h d) -> p b h d", b=NBLK, h=HEADS),
            in0=accR[:].rearrange("p (b w) -> p b w", w=CW)[:, :, 0:DIM]
                .rearrange("p b (h d) -> p b h d", h=HEADS),
            in1=rden_sb[:].rearrange("p (b h) -> p b h", h=HEADS)
                .unsqueeze(-1).to_broadcast([TILE, NBLK, HEADS, DH]),
            op=mult)
        aggT_sb = sp.tile([TILE, NSP], f32)
        nc.gpsimd.memset(aggT_sb[:], 0.0)
        for b in range(NBLK):
            tp_ps = pp.tile([DIM, TILE], f32, tag="aux")
            nc.tensor.transpose(out=tp_ps[:],
                                in_=agg_sb[:, b * TILE:(b + 1) * TILE],
                                identity=ID_sb[:])
            nc.scalar.copy(out=aggT_sb[:, b * TILE:(b + 1) * TILE], in_=tp_ps[:])
        for b in range(NSP // CHUNK):
            sl = slice(b * CHUNK, (b + 1) * CHUNK)
            out_ps = pp.tile([DIM, CHUNK], f32, tag="aux")
            nc.tensor.matmul(out=out_ps[:], lhsT=Wo_sb[:],
                             rhs=aggT_sb[:, sl], start=True, stop=True)
            osb = xp.tile([DIM, CHUNK], f32, tag="osb")
            nc.scalar.activation(out=osb[:], in_=out_ps[:],
                                 func=Ident, bias=bo_sb[:, 0:1])
            dmac(outT_d[:, sl], osb[:])

    nc.compile()
    return nc


# ---------------------------------------------------------------------------
# Entry point
# ---------------------------------------------------------------------------

def _prepare(inputs):
    q_nodes = np.asarray(inputs["q_nodes"], np.float32)
    k_edges = np.asarray(inputs["k_edges"], np.float32)
    v_edges = np.asarray(inputs["v_edges"], np.float32)
    Wq = np.asarray(inputs["Wq"], np.float32)
    bq = np.asarray(inputs["bq"], np.float32)
    Wk = np.asarray(inputs["Wk"], np.float32)
    Wv = np.asarray(inputs["Wv"], np.float32)
    bv = np.asarray(inputs["bv"], np.float32)
    Wo = np.asarray(inputs["Wo"], np.float32)
    bo = np.asarray(inputs["bo"], np.float32)
    dst = np.asarray(inputs["edge_index"])[0].astype(np.int64)

    plan = _make_plan(dst)

    eorder = np.argsort(dst, kind="stable")
    starts = np.zeros(N + 1, np.int64)
    np.cumsum(np.bincount(dst, minlength=N), out=starts[1:])
    edges_of = [eorder[starts[n]: starts[n + 1]] for n in range(N)]

    consts = {
        "Wq": Wq.astype(np.float16),
        "WkTs": np.ascontiguousarray((Wk * SCALE).T).astype(np.float16),
        "Wv": Wv.astype(np.float16),
        "Wo": np.ascontiguousarray(Wo),
        "Hm": (np.arange(DIM)[:, None] // DH == np.arange(HEADS)[None, :]
               ).astype(np.float16),
        "ID": np.eye(DIM, dtype=np.float32),
        "I4": np.eye(HEADS).astype(__import__('ml_dtypes').bfloat16),
        "bq": bq.reshape(DIM, 1).astype(np.float32),
        # sum(attn)==1 folds bv through Wo: out = (segv/den)@Wo + (bv@Wo + bo)
        "bo": (bv @ Wo + bo).reshape(DIM, 1).astype(np.float32),
    }
    return plan, dst, edges_of, consts, q_nodes, k_edges, v_edges, bo


def kernel(**inputs):
    from concourse.bass_utils import run_bass_kernel_spmd

    (plan, dst, edges_of, consts, q_nodes, k_edges, v_edges, bo) = _prepare(inputs)

    nc = _build_module(plan)

    in_maps = []
    slot_maps = []
    for c in range(NCORES):
        kvs, qT, qslot = _pack_core_inputs(plan, c, k_edges, v_edges,
                                           q_nodes, edges_of)
        m = {"kvs": kvs, "qT": qT}
        m.update(consts)
        in_maps.append(m)
        slot_maps.append(qslot)

    res = run_bass_kernel_spmd(nc, in_maps, core_ids=list(range(NCORES)))
    global LAST_RESULTS
    LAST_RESULTS = res

    out = np.zeros((N, DIM), np.float32)
    for c in range(NCORES):
        outT = res.results[c]["outT"]          # [DIM, nsp]
        qslot = slot_maps[c]
        valid = qslot >= 0
        out[qslot[valid]] = outT[:, : plan.nslot].T[valid]
    deg0 = plan.deg == 0
    if deg0.any():
        out[deg0] = bo
    return out



# revision 6
# speedup vs baseline: 1.6745x; 1.6745x over previous
"""Trainium2 Bass kernel for nn_NodeEdgeCrossAttention (v3).

Strategy (dst-sharded, zero-collective, fp16, minimal PE work):
  - Host sorts edges by destination node, greedily assigns nodes to 8 cores
    (balanced edge counts), and packs node edge-runs into 512-column chunks
    with a slot pattern shared by all cores (SPMD: one program).  No per-node
    padding: slot boundaries are arbitrary; chunk tails are zero-padded.
  - Scores fold Wq/Wk/scale into per-node M matrices computed on host (O(N)):
    score[e,h] = M[dst_e,h] . k_raw_e.  bk cancels by softmax shift
    invariance; bv folds through Wo into the bias since sum(attn)==1.
  - Wv folds into Wo per head (W2_h = Wv[:,h] @ Wo[h,:]), so the device
    aggregates RAW v vectors; no per-edge v projection on device.
  - Per 512-col chunk on device: 1 fused DMA; 4 score matmuls (k-tile
    stationary, M moving) -> [c=128, 4ns] PSUM per tile; 1 exp (scalar
    engine); 1 mask-multiply with the one-hot S (vector engine) -> exm;
    4 segment matmuls (exm stationary, [v|1] moving) accumulating
    [4ns, 129] in PSUM (weighted-v sums + softmax denominators);
    reciprocal of den (vector); normalize via scalar-engine per-partition
    scale -> fp16; small DMA to a DRAM scratch.
  - Final: one hardware transposing DMA (XBAR) of the whole scratch into
    SBUF, then 4 matmuls per 512-slot block with host-folded W2 + bias.
  - Numerics: fp16 linear tensors (fp8 fails the 2e-2 gate: attention-weight
    quantization error does not average down relative to the output), fp32
    accumulation, exp emits fp16 with a -ln16 bias that cancels in the
    normalize.
"""

import numpy as np

N, E, DIM, HEADS = 10000, 640000, 128, 4
DH = DIM // HEADS
NCORES = 8
CHUNK = 512
TILE = 128
SCALE = DH ** -0.5
NSMAX = 32                      # slots per chunk cap (PSUM: 16*ns <= 512 fp32)
LN16 = float(np.log(16.0))


class Plan:
    pass


def _make_plan(dst):
    """Greedy core assignment + shared chunk/slot pattern (no per-node pad)."""
    deg = np.bincount(dst, minlength=N)
    nz = np.where(deg > 0)[0]
    if deg.max() > CHUNK:
        raise NotImplementedError(f"max degree {deg.max()} > {CHUNK}")

    order = nz[np.argsort(-deg[nz], kind="stable")]
    loads = np.zeros(NCORES, np.int64)
    core_nodes = [[] for _ in range(NCORES)]
    for n in order:
        c = int(loads.argmin())
        core_nodes[c].append(int(n))
        loads[c] += deg[n]

    # Shared slot pattern: rank r -> max deg across cores at that rank.
    L = max(len(cn) for cn in core_nodes)
    pat = np.zeros(L, np.int64)
    for cn in core_nodes:
        d = deg[np.array(cn, np.int64)]
        pat[: len(d)] = np.maximum(pat[: len(d)], d)

    # First-fit decreasing bin packing of pattern slots into 512-col chunks.
    chunks = []                 # list of dict(slots=[(rank, col0, R)])
    open_rem = []               # remaining cols per open chunk
    for r in range(L):
        R = int(pat[r])
        placed = False
        for ci in range(len(chunks)):
            if open_rem[ci] >= R and len(chunks[ci]["slots"]) < NSMAX:
                col0 = CHUNK - open_rem[ci]
                chunks[ci]["slots"].append((r, col0, R))
                open_rem[ci] -= R
                placed = True
                break
        if not placed:
            chunks.append({"slots": [(r, 0, R)]})
            open_rem.append(CHUNK - R)

    # Global slot index g in (chunk, slot) order; map rank -> g.
    rank2g = np.full(L, -1, np.int64)
    g = 0
    for ch in chunks:
        ch["g0"] = g
        ch["ns"] = len(ch["slots"])
        for (r, _, _) in ch["slots"]:
            rank2g[r] = g
            g += 1

    p = Plan()
    p.deg = deg
    p.core_nodes = core_nodes
    p.chunks = chunks
    p.nchunks = len(chunks)
    p.L = L
    p.G = g                                   # total slots
    p.SLOTP = ((g + CHUNK - 1) // CHUNK) * CHUNK
    p.rank2g = rank2g
    # chunk fp16-element widths in the fused kvs tensor:
    # K(512) | V(4*129=516) | S(4ns) | M(4ns)
    p.offs = []
    off = 0
    for ch in chunks:
        ns = ch["ns"]
        w = 512 + 516 + 8 * ns
        p.offs.append(off)
        off += w
    p.KVW_total = off
    return p


def _pack_core_inputs(plan, c, kT_ext, v_ext, Mfull, edges_of):
    """Per-core fused kvs [128, KVW_total] fp16."""
    deg = plan.deg
    cn = plan.core_nodes[c]
    ncols = plan.nchunks * CHUNK
    idx = np.full(ncols, E, np.int64)          # E -> zero sentinel column
    for ch_i, ch in enumerate(plan.chunks):
        for (r, col0, R) in ch["slots"]:
            if r >= len(cn):
                continue
            node = cn[r]
            d = deg[node]
            g0 = ch_i * CHUNK + col0
            idx[g0: g0 + d] = edges_of[node]

    kT = kT_ext[:, idx]                        # [128, ncols] fp16
    vE = v_ext[idx]                            # [ncols, 128] fp16

    kvs = np.zeros((TILE, plan.KVW_total), np.float16)
    for ch_i, ch in enumerate(plan.chunks):
        ns = ch["ns"]
        off = plan.offs[ch_i]
        c0 = ch_i * CHUNK
        # K section [128 d, 512 c]
        kvs[:, off: off + 512] = kT[:, c0: c0 + CHUNK]
        # V section: 4 x [128 c, 128 d | 1]
        voff = off + 512
        for t in range(4):
            blk = kvs[:, voff + 129 * t: voff + 129 * t + 129]
            blk[:, 0:128] = vE[c0 + 128 * t: c0 + 128 * (t + 1)]
            blk[:, 128] = 1.0
        # S section: one-hot [128 c, (t, j)]
        soff = voff + 516
        for j, (r, col0, R) in enumerate(ch["slots"]):
            if r >= len(cn):
                continue
            d = int(deg[cn[r]])
            for t in range(4):
                lo = max(col0, t * TILE)
                hi = min(col0 + d, (t + 1) * TILE)
                if lo < hi:
                    kvs[lo - t * TILE: hi - t * TILE, soff + t * ns + j] = 1.0
        # M section: [128 d, (j, h)]
        moff = soff + 4 * ns
        for j, (r, col0, R) in enumerate(ch["slots"]):
            if r < len(cn):
                kvs[:, moff + 4 * j: moff + 4 * j + 4] = Mfull[:, cn[r], :]
    return kvs


# ---------------------------------------------------------------------------
# Device kernel emission
# ---------------------------------------------------------------------------

def _build_module(plan):
    import concourse.bacc as bacc
    import concourse.mybir as mybir
    import concourse.tile as tile
    from contextlib import ExitStack

    f16 = mybir.dt.float16
    f32 = mybir.dt.float32
    SLOTP = plan.SLOTP
    NBLK = SLOTP // CHUNK

    nc = bacc.Bacc("TRN2", debug=False, num_devices=NCORES)

    kvs_d = nc.dram_tensor("kvs", [TILE, plan.KVW_total], f16,
                           kind="ExternalInput")
    W2_d = nc.dram_tensor("W2", [DIM, 4 * DIM], f16, kind="ExternalInput")
    bo2_d = nc.dram_tensor("bo2", [DIM, 1], f32, kind="ExternalInput")
    scrD = nc.dram_tensor("scr", [4 * SLOTP, TILE], f16, kind="Internal")
    outT_d = nc.dram_tensor("outT", [DIM, SLOTP], f32, kind="ExternalOutput")

    Exp = mybir.ActivationFunctionType.Exp
    Ident = mybir.ActivationFunctionType.Identity
    mult = mybir.AluOpType.mult
    amax = mybir.AluOpType.max

    with ExitStack() as ctx:
        tc = ctx.enter_context(tile.TileContext(nc))
        cp = ctx.enter_context(tc.tile_pool(name="const", bufs=1))
        sp = ctx.enter_context(tc.tile_pool(name="persist", bufs=1))
        iop = ctx.enter_context(tc.tile_pool(name="io", bufs=4))
        xp = ctx.enter_context(tc.tile_pool(name="work", bufs=3))
        pp = ctx.enter_context(tc.tile_pool(name="ps", bufs=2, space="PSUM"))

        W2_sb = cp.tile([DIM, 4 * DIM], f16)
        nc.sync.dma_start(out=W2_sb[:], in_=W2_d[:, :])
        bo2_sb = cp.tile([DIM, 1], f32)
        nc.sync.dma_start(out=bo2_sb[:], in_=bo2_d[:, :])
        ln16_sb = cp.tile([DIM, 1], f32)
        nc.gpsimd.memset(ln16_sb[:], -LN16)

        for ci, ch in enumerate(plan.chunks):
            ns = ch["ns"]
            off = plan.offs[ci]
            W = 1028 + 8 * ns
            kvt = iop.tile([TILE, 1028 + 8 * NSMAX], f16, tag="kv")
            nc.sync.dma_start(out=kvt[:, 0:W], in_=kvs_d[:, off: off + W])
            ksec = kvt[:, 0:512]
            vsec = kvt[:, 512:1028]
            Ssec = kvt[:, 1028:1028 + 4 * ns]                  # [128, (t,j)]
            Msec = kvt[:, 1028 + 4 * ns: 1028 + 8 * ns]        # [128, (j,h)]

            # scores: per tile t, [c=128, (j,h)=4ns] = k_tile^T @ M
            score_ps = pp.tile([TILE, 16 * NSMAX], f32, tag="score")
            for t in range(4):
                nc.tensor.matmul(
                    out=score_ps[:, t * 4 * ns:(t + 1) * 4 * ns],
                    lhsT=ksec[:, t * TILE:(t + 1) * TILE],
                    rhs=Msec[:],
                    start=True, stop=True)

            # exp: exs = exp(score - ln16)  [128, 16ns] fp16
            exs = xp.tile([TILE, 16 * NSMAX], f16, tag="exs")
            nc.scalar.activation(out=exs[:, 0:16 * ns],
                                 in_=score_ps[:, 0:16 * ns],
                                 func=Exp, bias=ln16_sb[:, 0:1])

            # mask: exm = exs * S  (broadcast over h)
            exm = xp.tile([TILE, 16 * NSMAX], f16, tag="exm")
            nc.vector.tensor_tensor(
                out=exm[:, 0:16 * ns].rearrange("p (t j h) -> p t j h",
                                                t=4, h=4),
                in0=exs[:, 0:16 * ns].rearrange("p (t j h) -> p t j h",
                                                t=4, h=4),
                in1=Ssec[:].rearrange("p (t j) -> p t j", t=4)
                    .unsqueeze(-1).to_broadcast([TILE, 4, ns, 4]),
                op=mult)

            # segment matmuls: park[(j,h), 0:128]=sum exm*v ; [:,128]=den
            park = pp.tile([TILE, 129], f32, tag="park")
            for t in range(4):
                nc.tensor.matmul(
                    out=park[0:4 * ns, :],
                    lhsT=exm[:, t * 4 * ns:(t + 1) * 4 * ns],
                    rhs=vsec[:, 129 * t: 129 * t + 129],
                    start=(t == 0), stop=(t == 3))

            # normalize: rden = 1/max(den, eps); aggN = agg * rden (fp16)
            rdent = xp.tile([TILE, 2], f32, tag="rden")
            nc.vector.tensor_scalar(
                out=rdent[0:4 * ns, 0:1], in0=park[0:4 * ns, 128:129],
                scalar1=1e-30, scalar2=None, op0=amax)
            nc.vector.reciprocal(out=rdent[0:4 * ns, 1:2],
                                 in_=rdent[0:4 * ns, 0:1])
            aggN = xp.tile([TILE, TILE], f16, tag="aggN")
            nc.scalar.activation(out=aggN[0:4 * ns, :],
                                 in_=park[0:4 * ns, 0:128],
                                 func=Ident, scale=rdent[0:4 * ns, 1:2])

            g0 = ch["g0"]
            nc.scalar.dma_start(out=scrD[4 * g0: 4 * (g0 + ns), :],
                                in_=aggN[0:4 * ns, :])

        # ---- Final: transpose scratch, project with folded W2, bias ----
        stag = sp.tile([TILE, 4 * SLOTP], f16)
        nc.sync.dma_start_transpose(out=stag[:], in_=scrD[:, :])
        stag_r = stag[:].rearrange("p (s h) -> p s h", h=4)
        for b in range(NBLK):
            out_ps = pp.tile([DIM, CHUNK], f32, tag="out")
            for h in range(4):
                nc.tensor.matmul(
                    out=out_ps[:],
                    lhsT=W2_sb[:, h * DIM:(h + 1) * DIM],
                    rhs=stag_r[:, b * CHUNK:(b + 1) * CHUNK, h],
                    start=(h == 0), stop=(h == 3))
            osb = xp.tile([DIM, CHUNK], f32, tag="osb")
            nc.scalar.activation(out=osb[:], in_=out_ps[:],
                                 func=Ident, bias=bo2_sb[:, 0:1])
            nc.sync.dma_start(out=outT_d[:, b * CHUNK:(b + 1) * CHUNK],
                              in_=osb[:])

    nc.compile()
    return nc


# ---------------------------------------------------------------------------
# Entry point
# ---------------------------------------------------------------------------

def _prepare(inputs):
    q_nodes = np.asarray(inputs["q_nodes"], np.float32)
    k_edges = np.asarray(inputs["k_edges"], np.float32)
    v_edges = np.asarray(inputs["v_edges"], np.float32)
    Wq = np.asarray(inputs["Wq"], np.float32)
    bq = np.asarray(inputs["bq"], np.float32)
    Wk = np.asarray(inputs["Wk"], np.float32)
    Wv = np.asarray(inputs["Wv"], np.float32)
    bv = np.asarray(inputs["bv"], np.float32)
    Wo = np.asarray(inputs["Wo"], np.float32)
    bo = np.asarray(inputs["bo"], np.float32)
    dst = np.asarray(inputs["edge_index"])[0].astype(np.int64)

    plan = _make_plan(dst)

    eorder = np.argsort(dst, kind="stable")
    starts = np.zeros(N + 1, np.int64)
    np.cumsum(np.bincount(dst, minlength=N), out=starts[1:])
    edges_of = [eorder[starts[n]: starts[n + 1]] for n in range(N)]

    # host-side per-node score matrices M[d, n, h] and folded weights
    qp = q_nodes @ Wq + bq
    Mfull = np.empty((DIM, N, HEADS), np.float32)
    for h in range(HEADS):
        sl = slice(h * DH, (h + 1) * DH)
        Mfull[:, :, h] = (Wk[:, sl] * SCALE) @ qp[:, sl].T
    Mfull = Mfull.astype(np.float16)

    W2 = np.empty((DIM, 4 * DIM), np.float32)
    for h in range(HEADS):
        sl = slice(h * DH, (h + 1) * DH)
        W2[:, h * DIM:(h + 1) * DIM] = Wv[:, sl] @ Wo[sl, :]
    consts = {
        "W2": W2.astype(np.float16),
        "bo2": (bv @ Wo + bo).reshape(DIM, 1).astype(np.float32),
    }

    kT_ext = np.zeros((DIM, E + 1), np.float16)
    kT_ext[:, :E] = k_edges.T.astype(np.float16)
    v_ext = np.zeros((E + 1, DIM), np.float16)
    v_ext[:E] = v_edges.astype(np.float16)

    return plan, edges_of, consts, kT_ext, v_ext, Mfull, bo


def kernel(**inputs):
    from concourse.bass_utils import run_bass_kernel_spmd

    plan, edges_of, consts, kT_ext, v_ext, Mfull, bo = _prepare(inputs)

    nc = _build_module(plan)

    in_maps = []
    for c in range(NCORES):
        kvs = _pack_core_inputs(plan, c, kT_ext, v_ext, Mfull, edges_of)
        m = {"kvs": kvs}
        m.update(consts)
        in_maps.append(m)

    res = run_bass_kernel_spmd(nc, in_maps, core_ids=list(range(NCORES)))
    global LAST_RESULTS
    LAST_RESULTS = res

    out = np.zeros((N, DIM), np.float32)
    for c in range(NCORES):
        outT = res.results[c]["outT"]              # [128, SLOTP]
        cn = plan.core_nodes[c]
        gs = plan.rank2g[: len(cn)]
        out[np.array(cn, np.int64)] = outT[:, gs].T
    deg0 = plan.deg == 0
    if deg0.any():
        out[deg0] = bo
    return out


# revision 9
# speedup vs baseline: 2.1538x; 1.2862x over previous
"""Trainium2 Bass kernel for nn_NodeEdgeCrossAttention (v3).

Strategy (dst-sharded, zero-collective, fp16, minimal PE work):
  - Host sorts edges by destination node, greedily assigns nodes to 8 cores
    (balanced edge counts), and packs node edge-runs into 1024-column chunks
    with a slot pattern shared by all cores (SPMD: one program).  No per-node
    padding: slot boundaries are arbitrary; chunk tails are zero-padded.
  - Scores fold Wq/Wk/scale into per-node M matrices computed on host (O(N)):
    score[e,h] = M[dst_e,h] . k_raw_e.  bk cancels by softmax shift
    invariance; bv folds through Wo into the bias since sum(attn)==1.
  - Wv folds into Wo per head (W2_h = Wv[:,h] @ Wo[h,:]), so the device
    aggregates RAW v vectors; no per-edge v projection on device.
  - Per chunk on device: 1 fused DMA; NT score matmuls (k-tile
    stationary, M moving) -> [c=128, 4ns] PSUM per tile; 1 exp (scalar
    engine); 1 mask-multiply with the one-hot S (vector engine) -> exm;

    NT segment matmuls (exm stationary, [v|1] moving) accumulating
    [4ns, 129] in PSUM (weighted-v sums + softmax denominators);
    reciprocal of den + normalize multiply (vector) -> fp16; small DMA
    to a DRAM scratch.
  - Final: one hardware transposing DMA (XBAR) of the whole scratch into
    SBUF, then 4 matmuls per 512-slot block with host-folded W2 + bias.
  - Numerics: fp16 linear tensors (fp8 fails the 2e-2 gate: attention-weight
    quantization error does not average down relative to the output), fp32
    accumulation, exp emits fp16 with a -ln16 bias that cancels in the
    normalize.
"""

import numpy as np

N, E, DIM, HEADS = 10000, 640000, 128, 4
DH = DIM // HEADS
NCORES = 8
CHUNK = 1024
NT = CHUNK // 128                 # k/v tiles per chunk
TILE = 128
SCALE = DH ** -0.5
NSMAX = 32                      # slots per chunk cap (PSUM: 4*NT*ns <= 1024 fp32)
LN16 = float(np.log(16.0))


class Plan:
    pass


def _make_plan(dst):
    """Greedy core assignment + shared chunk/slot pattern (no per-node pad)."""
    deg = np.bincount(dst, minlength=N)
    nz = np.where(deg > 0)[0]
    if deg.max() > CHUNK:
        raise NotImplementedError(f"max degree {deg.max()} > {CHUNK}")

    order = nz[np.argsort(-deg[nz], kind="stable")]
    loads = np.zeros(NCORES, np.int64)
    core_nodes = [[] for _ in range(NCORES)]
    for n in order:
        c = int(loads.argmin())
        core_nodes[c].append(int(n))
        loads[c] += deg[n]

    # Shared slot pattern: rank r -> max deg across cores at that rank.
    L = max(len(cn) for cn in core_nodes)
    pat = np.zeros(L, np.int64)
    for cn in core_nodes:
        d = deg[np.array(cn, np.int64)]
        pat[: len(d)] = np.maximum(pat[: len(d)], d)

    # First-fit decreasing bin packing of pattern slots into 512-col chunks.
    chunks = []                 # list of dict(slots=[(rank, col0, R)])
    open_rem = []               # remaining cols per open chunk
    for r in range(L):
        R = int(pat[r])
        placed = False
        for ci in range(len(chunks)):
            if open_rem[ci] >= R and len(chunks[ci]["slots"]) < NSMAX:
                col0 = CHUNK - open_rem[ci]
                chunks[ci]["slots"].append((r, col0, R))
                open_rem[ci] -= R
                placed = True
                break
        if not placed:
            chunks.append({"slots": [(r, 0, R)]})
            open_rem.append(CHUNK - R)

    # Global slot index g in (chunk, slot) order; map rank -> g.
    rank2g = np.full(L, -1, np.int64)
    g = 0
    for ch in chunks:
        ch["g0"] = g
        ch["ns"] = len(ch["slots"])
        for (r, _, _) in ch["slots"]:
            rank2g[r] = g
            g += 1

    p = Plan()
    p.deg = deg
    p.core_nodes = core_nodes
    p.chunks = chunks
    p.nchunks = len(chunks)
    p.L = L
    p.G = g                                   # total slots
    p.SLOTP = ((g + 511) // 512) * 512
    p.rank2g = rank2g
    # chunk fp16-element widths in the fused kvs tensor:
    # K(CHUNK) | V(NT*129) | S(NT*ns) | M(4ns)
    p.offs = []
    off = 0
    for ch in chunks:
        ns = ch["ns"]
        w = CHUNK + 129 * NT + (NT + 4) * ns
        p.offs.append(off)
        off += w
    p.KVW_total = off
    return p


def _pack_core_inputs(plan, c, kT_ext, v_ext, Mfull, edges_of):
    """Per-core fused kvs [128, KVW_total] fp16."""
    deg = plan.deg
    cn = plan.core_nodes[c]
    ncols = plan.nchunks * CHUNK
    idx = np.full(ncols, E, np.int64)          # E -> zero sentinel column
    for ch_i, ch in enumerate(plan.chunks):
        for (r, col0, R) in ch["slots"]:
            if r >= len(cn):
                continue
            node = cn[r]
            d = deg[node]
            g0 = ch_i * CHUNK + col0
            idx[g0: g0 + d] = edges_of[node]

    kT = kT_ext[:, idx]                        # [128, ncols] fp16
    vE = v_ext[idx]                            # [ncols, 128] fp16

    kvs = np.zeros((TILE, plan.KVW_total), np.float16)
    for ch_i, ch in enumerate(plan.chunks):
        ns = ch["ns"]
        off = plan.offs[ch_i]
        c0 = ch_i * CHUNK
        # K section [128 d, CHUNK c]
        kvs[:, off: off + CHUNK] = kT[:, c0: c0 + CHUNK]
        # V section: NT x [128 c, 128 d | 1]
        voff = off + CHUNK
        for t in range(NT):
            blk = kvs[:, voff + 129 * t: voff + 129 * t + 129]
            blk[:, 0:128] = vE[c0 + 128 * t: c0 + 128 * (t + 1)]
            blk[:, 128] = 1.0
        # S section: one-hot [128 c, (t, j)]
        soff = voff + 129 * NT
        for j, (r, col0, R) in enumerate(ch["slots"]):
            if r >= len(cn):
                continue
            d = int(deg[cn[r]])
            for t in range(NT):
                lo = max(col0, t * TILE)
                hi = min(col0 + d, (t + 1) * TILE)
                if lo < hi:
                    kvs[lo - t * TILE: hi - t * TILE, soff + t * ns + j] = 1.0
        # M section: [128 d, (j, h)]
        moff = soff + NT * ns
        for j, (r, col0, R) in enumerate(ch["slots"]):
            if r < len(cn):
                kvs[:, moff + 4 * j: moff + 4 * j + 4] = Mfull[:, cn[r], :]
    return kvs


# ---------------------------------------------------------------------------
# Device kernel emission
# ---------------------------------------------------------------------------

def _build_module(plan):
    import concourse.bacc as bacc
    import concourse.mybir as mybir
    import concourse.tile as tile
    from contextlib import ExitStack

    f16 = mybir.dt.float16
    f32 = mybir.dt.float32
    SLOTP = plan.SLOTP
    NBLK = SLOTP // 512

    nc = bacc.Bacc("TRN2", debug=False, num_devices=NCORES)

    kvs_d = nc.dram_tensor("kvs", [TILE, plan.KVW_total], f16,
                           kind="ExternalInput")
    W2_d = nc.dram_tensor("W2", [DIM, 4 * DIM], f16, kind="ExternalInput")
    bo2_d = nc.dram_tensor("bo2", [DIM, 1], f32, kind="ExternalInput")
    scrD = nc.dram_tensor("scr", [4 * SLOTP, TILE], f16, kind="Internal")
    outT_d = nc.dram_tensor("outT", [DIM, SLOTP], f32, kind="ExternalOutput")

    Exp = mybir.ActivationFunctionType.Exp
    Ident = mybir.ActivationFunctionType.Identity
    mult = mybir.AluOpType.mult
    amax = mybir.AluOpType.max

    with ExitStack() as ctx:
        tc = ctx.enter_context(tile.TileContext(nc))
        cp = ctx.enter_context(tc.tile_pool(name="const", bufs=1))
        sp = ctx.enter_context(tc.tile_pool(name="persist", bufs=1))
        iop = ctx.enter_context(tc.tile_pool(name="io", bufs=6))
        xp = ctx.enter_context(tc.tile_pool(name="work", bufs=4))
        pp = ctx.enter_context(tc.tile_pool(name="ps", bufs=2, space="PSUM"))

        W2_sb = cp.tile([DIM, 4 * DIM], f16)
        nc.sync.dma_start(out=W2_sb[:], in_=W2_d[:, :])
        bo2_sb = cp.tile([DIM, 1], f32)
        nc.sync.dma_start(out=bo2_sb[:], in_=bo2_d[:, :])
        ln16_sb = cp.tile([DIM, 1], f32)
        nc.gpsimd.memset(ln16_sb[:], -LN16)

        VOFF = CHUNK
        SOFF = CHUNK + 129 * NT
        for ci, ch in enumerate(plan.chunks):
            ns = ch["ns"]
            off = plan.offs[ci]
            W = SOFF + (NT + 4) * ns
            kvt = iop.tile([TILE, SOFF + (NT + 4) * NSMAX], f16, tag="kv")
            nc.sync.dma_start(out=kvt[:, 0:W], in_=kvs_d[:, off: off + W])
            ksec = kvt[:, 0:CHUNK]
            vsec = kvt[:, VOFF:SOFF]
            Ssec = kvt[:, SOFF:SOFF + NT * ns]                 # [128, (t,j)]
            Msec = kvt[:, SOFF + NT * ns: SOFF + (NT + 4) * ns]  # [128, (j,h)]

            # scores: per tile t, [c=128, (j,h)=4ns] = k_tile^T @ M.
            # Each tile gets a fixed 128-col stride in PSUM so no matmul
            # output ever crosses a 2KB PSUM bank boundary (4ns <= 128).
            score_ps = pp.tile([TILE, NT * TILE], f32, tag="score")
            for t in range(NT):
                nc.tensor.matmul(
                    out=score_ps[:, t * TILE: t * TILE + 4 * ns],
                    lhsT=ksec[:, t * TILE:(t + 1) * TILE],
                    rhs=Msec[:],
                    start=True, stop=True)

            # exp: exs = exp(score - ln16), strided over real cols only
            exs = xp.tile([TILE, NT * TILE], f16, tag="exs")
            score_r = score_ps[:].rearrange("p (t c) -> p t c", t=NT)
            exs_r = exs[:].rearrange("p (t c) -> p t c", t=NT)
            nc.scalar.activation(out=exs_r[:, :, 0:4 * ns],
                                 in_=score_r[:, :, 0:4 * ns],
                                 func=Exp, bias=ln16_sb[:, 0:1])

            # mask: exm = exs * S  (broadcast over h)
            exm = xp.tile([TILE, NT * TILE], f16, tag="exm")
            exm_r = exm[:].rearrange("p (t c) -> p t c", t=NT)
            nc.vector.tensor_tensor(
                out=exm_r[:, :, 0:4 * ns].rearrange("p t (j h) -> p t j h",
                                                    h=4),
                in0=exs_r[:, :, 0:4 * ns].rearrange("p t (j h) -> p t j h",
                                                    h=4),
                in1=Ssec[:].rearrange("p (t j) -> p t j", t=NT)
                    .unsqueeze(-1).to_broadcast([TILE, NT, ns, 4]),
                op=mult)

            # segment matmuls: park[(j,h), 0:128]=sum exm*v ; [:,128]=den
            park = pp.tile([TILE, 129], f32, tag="park")
            for t in range(NT):
                nc.tensor.matmul(
                    out=park[0:4 * ns, :],
                    lhsT=exm[:, t * TILE: t * TILE + 4 * ns],
                    rhs=vsec[:, 129 * t: 129 * t + 129],
                    start=(t == 0), stop=(t == NT - 1))

            # normalize: rden = 1/den (filler slots: den=0 -> discarded NaN)
            rdent = xp.tile([TILE, 1], f32, tag="rden")
            nc.vector.reciprocal(out=rdent[0:4 * ns, 0:1],
                                 in_=park[0:4 * ns, 128:129])
            aggN = xp.tile([TILE, TILE], f16, tag="aggN")
            nc.vector.tensor_tensor(
                out=aggN[0:4 * ns, :],
                in0=park[0:4 * ns, 0:128],
                in1=rdent[0:4 * ns, 0:1].to_broadcast([4 * ns, TILE]),
                op=mult)

            g0 = ch["g0"]
            nc.scalar.dma_start(out=scrD[4 * g0: 4 * (g0 + ns), :],
                                in_=aggN[0:4 * ns, :])

        # ---- Final: transpose scratch, project with folded W2, bias ----
        stag = sp.tile([TILE, 4 * SLOTP], f16)
        nc.sync.dma_start_transpose(out=stag[:], in_=scrD[:, :])
        stag_r = stag[:].rearrange("p (s h) -> p s h", h=4)
        for b in range(NBLK):
            out_ps = pp.tile([DIM, 512], f32, tag="out")
            for h in range(4):
                nc.tensor.matmul(
                    out=out_ps[:],
                    lhsT=W2_sb[:, h * DIM:(h + 1) * DIM],
                    rhs=stag_r[:, b * 512:(b + 1) * 512, h],
                    start=(h == 0), stop=(h == 3))
            osb = xp.tile([DIM, 512], f32, tag="osb")
            nc.scalar.activation(out=osb[:], in_=out_ps[:],
                                 func=Ident, bias=bo2_sb[:, 0:1])
            nc.sync.dma_start(out=outT_d[:, b * 512:(b + 1) * 512],
                              in_=osb[:])

    nc.compile()
    return nc


# ---------------------------------------------------------------------------
# Entry point
# ---------------------------------------------------------------------------

def _prepare(inputs):
    q_nodes = np.asarray(inputs["q_nodes"], np.float32)
    k_edges = np.asarray(inputs["k_edges"], np.float32)
    v_edges = np.asarray(inputs["v_edges"], np.float32)
    Wq = np.asarray(inputs["Wq"], np.float32)
    bq = np.asarray(inputs["bq"], np.float32)
    Wk = np.asarray(inputs["Wk"], np.float32)
    Wv = np.asarray(inputs["Wv"], np.float32)
    bv = np.asarray(inputs["bv"], np.float32)
    Wo = np.asarray(inputs["Wo"], np.float32)
    bo = np.asarray(inputs["bo"], np.float32)
    dst = np.asarray(inputs["edge_index"])[0].astype(np.int64)

    plan = _make_plan(dst)

    eorder = np.argsort(dst, kind="stable")
    starts = np.zeros(N + 1, np.int64)
    np.cumsum(np.bincount(dst, minlength=N), out=starts[1:])
    edges_of = [eorder[starts[n]: starts[n + 1]] for n in range(N)]

    # host-side per-node score matrices M[d, n, h] and folded weights
    qp = q_nodes @ Wq + bq
    Mfull = np.empty((DIM, N, HEADS), np.float32)
    for h in range(HEADS):
        sl = slice(h * DH, (h + 1) * DH)
        Mfull[:, :, h] = (Wk[:, sl] * SCALE) @ qp[:, sl].T
    Mfull = Mfull.astype(np.float16)

    W2 = np.empty((DIM, 4 * DIM), np.float32)
    for h in range(HEADS):
        sl = slice(h * DH, (h + 1) * DH)
        W2[:, h * DIM:(h + 1) * DIM] = Wv[:, sl] @ Wo[sl, :]
    consts = {
        "W2": W2.astype(np.float16),
        "bo2": (bv @ Wo + bo).reshape(DIM, 1).astype(np.float32),
    }

    kT_ext = np.zeros((DIM, E + 1), np.float16)
    kT_ext[:, :E] = k_edges.T.astype(np.float16)
    v_ext = np.zeros((E + 1, DIM), np.float16)
    v_ext[:E] = v_edges.astype(np.float16)

    return plan, edges_of, consts, kT_ext, v_ext, Mfull, bo


def kernel(**inputs):
    from concourse.bass_utils import run_bass_kernel_spmd

    plan, edges_of, consts, kT_ext, v_ext, Mfull, bo = _prepare(inputs)

    nc = _build_module(plan)

    in_maps = []
    for c in range(NCORES):
        kvs = _pack_core_inputs(plan, c, kT_ext, v_ext, Mfull, edges_of)
        m = {"kvs": kvs}
        m.update(consts)
        in_maps.append(m)

    res = run_bass_kernel_spmd(nc, in_maps, core_ids=list(range(NCORES)))
    global LAST_RESULTS
    LAST_RESULTS = res

    out = np.zeros((N, DIM), np.float32)
    for c in range(NCORES):
        outT = res.results[c]["outT"]              # [128, SLOTP]
        cn = plan.core_nodes[c]
        gs = plan.rank2g[: len(cn)]
        out[np.array(cn, np.int64)] = outT[:, gs].T
    deg0 = plan.deg == 0
    if deg0.any():
        out[deg0] = bo
    return out


# revision 10
# speedup vs baseline: 2.2022x; 1.0225x over previous
"""Trainium2 Bass kernel for nn_NodeEdgeCrossAttention (v3).

Strategy (dst-sharded, zero-collective, fp16, minimal PE work):
  - Host sorts edges by destination node, greedily assigns nodes to 8 cores
    (balanced edge counts), and packs node edge-runs into 1024-column chunks
    with a slot pattern shared by all cores (SPMD: one program).  No per-node
    padding: slot boundaries are arbitrary; chunk tails are zero-padded.
  - Scores fold Wq/Wk/scale into per-node M matrices computed on host (O(N)):
    score[e,h] = M[dst_e,h] . k_raw_e.  bk cancels by softmax shift
    invariance; bv folds through Wo into the bias since sum(attn)==1.
  - Wv folds into Wo per head (W2_h = Wv[:,h] @ Wo[h,:]), so the device
    aggregates RAW v vectors; no per-edge v projection on device.
  - Per chunk on device: 1 fused DMA; NT score matmuls (k-tile
    stationary, M moving) -> [c=128, 4ns] PSUM per tile; 1 exp (scalar
    engine); 1 mask-multiply with the one-hot S (vector engine) -> exm;

    NT segment matmuls (exm stationary, [v|1] moving) accumulating
    [4ns, 129] in PSUM (weighted-v sums + softmax denominators);
    reciprocal of den + normalize multiply (vector) -> fp16; PE transpose
    into a persistent SBUF staging buffer.
  - Output blocks of 512 slots are projected with host-folded W2 + bias as
    soon as their slots are staged (overlapped with the chunk stream).
  - Numerics: fp16 linear tensors (fp8 fails the 2e-2 gate: attention-weight
    quantization error does not average down relative to the output), fp32
    accumulation, exp emits fp16 with a -ln16 bias that cancels in the
    normalize.
"""

import numpy as np

N, E, DIM, HEADS = 10000, 640000, 128, 4
DH = DIM // HEADS
NCORES = 8
CHUNK = 1024
NT = CHUNK // 128                 # k/v tiles per chunk
TILE = 128
SCALE = DH ** -0.5
NSMAX = 32                      # slots per chunk cap (PSUM: 4*NT*ns <= 1024 fp32)
LN16 = float(np.log(16.0))


class Plan:
    pass


def _make_plan(dst):
    """Greedy core assignment + shared chunk/slot pattern (no per-node pad)."""
    deg = np.bincount(dst, minlength=N)
    nz = np.where(deg > 0)[0]
    if deg.max() > CHUNK:
        raise NotImplementedError(f"max degree {deg.max()} > {CHUNK}")

    order = nz[np.argsort(-deg[nz], kind="stable")]
    loads = np.zeros(NCORES, np.int64)
    core_nodes = [[] for _ in range(NCORES)]
    for n in order:
        c = int(loads.argmin())
        core_nodes[c].append(int(n))
        loads[c] += deg[n]

    # Shared slot pattern: rank r -> max deg across cores at that rank.
    L = max(len(cn) for cn in core_nodes)
    pat = np.zeros(L, np.int64)
    for cn in core_nodes:
        d = deg[np.array(cn, np.int64)]
        pat[: len(d)] = np.maximum(pat[: len(d)], d)

    # First-fit decreasing bin packing of pattern slots into 512-col chunks.
    chunks = []                 # list of dict(slots=[(rank, col0, R)])
    open_rem = []               # remaining cols per open chunk
    for r in range(L):
        R = int(pat[r])
        placed = False
        for ci in range(len(chunks)):
            if open_rem[ci] >= R and len(chunks[ci]["slots"]) < NSMAX:
                col0 = CHUNK - open_rem[ci]
                chunks[ci]["slots"].append((r, col0, R))
                open_rem[ci] -= R
                placed = True
                break
        if not placed:
            chunks.append({"slots": [(r, 0, R)]})
            open_rem.append(CHUNK - R)

    # Global slot index g in (chunk, slot) order; map rank -> g.
    rank2g = np.full(L, -1, np.int64)
    g = 0
    for ch in chunks:
        ch["g0"] = g
        ch["ns"] = len(ch["slots"])
        for (r, _, _) in ch["slots"]:
            rank2g[r] = g
            g += 1

    p = Plan()
    p.deg = deg
    p.core_nodes = core_nodes
    p.chunks = chunks
    p.nchunks = len(chunks)
    p.L = L
    p.G = g                                   # total slots
    p.SLOTP = ((g + 511) // 512) * 512
    p.rank2g = rank2g
    # chunk fp16-element widths in the fused kvs tensor:
    # K(CHUNK) | V(NT*129) | S(NT*ns) | M(4ns)
    p.offs = []
    off = 0
    for ch in chunks:
        ns = ch["ns"]
        w = CHUNK + 129 * NT + (NT + 4) * ns
        p.offs.append(off)
        off += w
    p.KVW_total = off
    return p


def _pack_core_inputs(plan, c, kT_ext, v_ext, Mfull, edges_of):
    """Per-core fused kvs [128, KVW_total] fp16."""
    deg = plan.deg
    cn = plan.core_nodes[c]
    ncols = plan.nchunks * CHUNK
    idx = np.full(ncols, E, np.int64)          # E -> zero sentinel column
    for ch_i, ch in enumerate(plan.chunks):
        for (r, col0, R) in ch["slots"]:
            if r >= len(cn):
                continue
            node = cn[r]
            d = deg[node]
            g0 = ch_i * CHUNK + col0
            idx[g0: g0 + d] = edges_of[node]

    kT = kT_ext[:, idx]                        # [128, ncols] fp16
    vE = v_ext[idx]                            # [ncols, 128] fp16

    kvs = np.zeros((TILE, plan.KVW_total), np.float16)
    for ch_i, ch in enumerate(plan.chunks):
        ns = ch["ns"]
        off = plan.offs[ch_i]
        c0 = ch_i * CHUNK
        # K section [128 d, CHUNK c]
        kvs[:, off: off + CHUNK] = kT[:, c0: c0 + CHUNK]
        # V section: NT x [128 c, 128 d | 1]
        voff = off + CHUNK
        for t in range(NT):
            blk = kvs[:, voff + 129 * t: voff + 129 * t + 129]
            blk[:, 0:128] = vE[c0 + 128 * t: c0 + 128 * (t + 1)]
            blk[:, 128] = 1.0
        # S section: one-hot [128 c, (t, j)]
        soff = voff + 129 * NT
        for j, (r, col0, R) in enumerate(ch["slots"]):
            if r >= len(cn):
                continue
            d = int(deg[cn[r]])
            for t in range(NT):
                lo = max(col0, t * TILE)
                hi = min(col0 + d, (t + 1) * TILE)
                if lo < hi:
                    kvs[lo - t * TILE: hi - t * TILE, soff + t * ns + j] = 1.0
        # M section: [128 d, (j, h)]
        moff = soff + NT * ns
        for j, (r, col0, R) in enumerate(ch["slots"]):
            if r < len(cn):
                kvs[:, moff + 4 * j: moff + 4 * j + 4] = Mfull[:, cn[r], :]
    return kvs


# ---------------------------------------------------------------------------
# Device kernel emission
# ---------------------------------------------------------------------------

def _build_module(plan):
    import concourse.bacc as bacc
    import concourse.mybir as mybir
    import concourse.tile as tile
    from contextlib import ExitStack

    f16 = mybir.dt.float16
    f32 = mybir.dt.float32
    SLOTP = plan.SLOTP
    NBLK = SLOTP // 512

    nc = bacc.Bacc("TRN2", debug=False, num_devices=NCORES)

    kvs_d = nc.dram_tensor("kvs", [TILE, plan.KVW_total], f16,
                           kind="ExternalInput")
    W2_d = nc.dram_tensor("W2", [DIM, 4 * DIM], f16, kind="ExternalInput")
    bo2_d = nc.dram_tensor("bo2", [DIM, 1], f32, kind="ExternalInput")
    ID_d = nc.dram_tensor("ID", [DIM, DIM], f16, kind="ExternalInput")
    outT_d = nc.dram_tensor("outT", [DIM, SLOTP], f32, kind="ExternalOutput")

    Exp = mybir.ActivationFunctionType.Exp
    Ident = mybir.ActivationFunctionType.Identity
    mult = mybir.AluOpType.mult
    amax = mybir.AluOpType.max

    with ExitStack() as ctx:
        tc = ctx.enter_context(tile.TileContext(nc))
        cp = ctx.enter_context(tc.tile_pool(name="const", bufs=1))
        sp = ctx.enter_context(tc.tile_pool(name="persist", bufs=1))
        iop = ctx.enter_context(tc.tile_pool(name="io", bufs=8))
        xp = ctx.enter_context(tc.tile_pool(name="work", bufs=4))
        pp = ctx.enter_context(tc.tile_pool(name="ps", bufs=2, space="PSUM"))

        W2_sb = cp.tile([DIM, 4 * DIM], f16)
        nc.sync.dma_start(out=W2_sb[:], in_=W2_d[:, :])
        bo2_sb = cp.tile([DIM, 1], f32)
        nc.sync.dma_start(out=bo2_sb[:], in_=bo2_d[:, :])
        ln16_sb = cp.tile([DIM, 1], f32)
        nc.gpsimd.memset(ln16_sb[:], -LN16)
        ID_sb = cp.tile([DIM, DIM], f16)
        nc.sync.dma_start(out=ID_sb[:], in_=ID_d[:, :])
        stag = sp.tile([TILE, 4 * SLOTP], f16)
        stag_r = stag[:].rearrange("p (s h) -> p s h", h=4)

        def emit_block(b):
            out_ps = pp.tile([TILE, NT * TILE], f32, tag="score")
            for h in range(4):
                nc.tensor.matmul(
                    out=out_ps[:, 0:512],
                    lhsT=W2_sb[:, h * DIM:(h + 1) * DIM],
                    rhs=stag_r[:, b * 512:(b + 1) * 512, h],
                    start=(h == 0), stop=(h == 3))
            osb = xp.tile([DIM, 512], f32, tag="osb")
            nc.scalar.activation(out=osb[:], in_=out_ps[:, 0:512],
                                 func=Ident, bias=bo2_sb[:, 0:1])
            nc.sync.dma_start(out=outT_d[:, b * 512:(b + 1) * 512],
                              in_=osb[:])

        next_block = 0

        VOFF = CHUNK
        SOFF = CHUNK + 129 * NT
        for ci, ch in enumerate(plan.chunks):
            ns = ch["ns"]
            off = plan.offs[ci]
            W = SOFF + (NT + 4) * ns
            kvt = iop.tile([TILE, SOFF + (NT + 4) * NSMAX], f16, tag="kv")
            nc.sync.dma_start(out=kvt[:, 0:W], in_=kvs_d[:, off: off + W])
            ksec = kvt[:, 0:CHUNK]
            vsec = kvt[:, VOFF:SOFF]
            Ssec = kvt[:, SOFF:SOFF + NT * ns]                 # [128, (t,j)]
            Msec = kvt[:, SOFF + NT * ns: SOFF + (NT + 4) * ns]  # [128, (j,h)]

            # scores: per tile t, [c=128, (j,h)=4ns] = k_tile^T @ M.
            # Each tile gets a fixed 128-col stride in PSUM so no matmul
            # output ever crosses a 2KB PSUM bank boundary (4ns <= 128).
            score_ps = pp.tile([TILE, NT * TILE], f32, tag="score")
            for t in range(NT):
                nc.tensor.matmul(
                    out=score_ps[:, t * TILE: t * TILE + 4 * ns],
                    lhsT=ksec[:, t * TILE:(t + 1) * TILE],
                    rhs=Msec[:],
                    start=True, stop=True)

            # exp: exs = exp(score - ln16), strided over real cols only
            exs = xp.tile([TILE, NT * TILE], f16, tag="exs")
            score_r = score_ps[:].rearrange("p (t c) -> p t c", t=NT)
            exs_r = exs[:].rearrange("p (t c) -> p t c", t=NT)
            nc.scalar.activation(out=exs_r[:, :, 0:4 * ns],
                                 in_=score_r[:, :, 0:4 * ns],
                                 func=Exp, bias=ln16_sb[:, 0:1])

            # mask: exm = exs * S  (broadcast over h)
            exm = xp.tile([TILE, NT * TILE], f16, tag="exm")
            exm_r = exm[:].rearrange("p (t c) -> p t c", t=NT)
            nc.vector.tensor_tensor(
                out=exm_r[:, :, 0:4 * ns].rearrange("p t (j h) -> p t j h",
                                                    h=4),
                in0=exs_r[:, :, 0:4 * ns].rearrange("p t (j h) -> p t j h",
                                                    h=4),
                in1=Ssec[:].rearrange("p (t j) -> p t j", t=NT)
                    .unsqueeze(-1).to_broadcast([TILE, NT, ns, 4]),
                op=mult)

            # segment matmuls: park[(j,h), 0:128]=sum exm*v ; [:,128]=den
            park = pp.tile([TILE, 129], f32, tag="park")
            for t in range(NT):
                nc.tensor.matmul(
                    out=park[0:4 * ns, :],
                    lhsT=exm[:, t * TILE: t * TILE + 4 * ns],
                    rhs=vsec[:, 129 * t: 129 * t + 129],
                    start=(t == 0), stop=(t == NT - 1))

            # normalize: rden = 1/den (filler slots: den=0 -> discarded NaN)
            rdent = xp.tile([TILE, 1], f32, tag="rden")
            nc.vector.reciprocal(out=rdent[0:4 * ns, 0:1],
                                 in_=park[0:4 * ns, 128:129])
            aggN = xp.tile([TILE, TILE], f16, tag="aggN")
            nc.vector.tensor_tensor(
                out=aggN[0:4 * ns, :],
                in0=park[0:4 * ns, 0:128],
                in1=rdent[0:4 * ns, 0:1].to_broadcast([4 * ns, TILE]),
                op=mult)

            # transpose on PE and stage in SBUF (no DRAM roundtrip)
            g0 = ch["g0"]
            tp_ps = pp.tile([TILE, TILE], f16, tag="tp")
            nc.tensor.transpose(out=tp_ps[0:TILE, 0:4 * ns],
                                in_=aggN[0:4 * ns, :],
                                identity=ID_sb[0:4 * ns, 0:4 * ns])
            nc.scalar.copy(out=stag[:, 4 * g0: 4 * (g0 + ns)],
                           in_=tp_ps[0:TILE, 0:4 * ns])

            # emit any output block whose slots are fully staged
            while next_block < NBLK and (g0 + ns) * 4 >= (next_block + 1) * 2048:
                emit_block(next_block)
                next_block += 1

        while next_block < NBLK:
            emit_block(next_block)
            next_block += 1

    nc.compile()
    return nc


# ---------------------------------------------------------------------------
# Entry point
# ---------------------------------------------------------------------------

def _prepare(inputs):
    q_nodes = np.asarray(inputs["q_nodes"], np.float32)
    k_edges = np.asarray(inputs["k_edges"], np.float32)
    v_edges = np.asarray(inputs["v_edges"], np.float32)
    Wq = np.asarray(inputs["Wq"], np.float32)
    bq = np.asarray(inputs["bq"], np.float32)
    Wk = np.asarray(inputs["Wk"], np.float32)
    Wv = np.asarray(inputs["Wv"], np.float32)
    bv = np.asarray(inputs["bv"], np.float32)
    Wo = np.asarray(inputs["Wo"], np.float32)
    bo = np.asarray(inputs["bo"], np.float32)
    dst = np.asarray(inputs["edge_index"])[0].astype(np.int64)

    plan = _make_plan(dst)

    eorder = np.argsort(dst, kind="stable")
    starts = np.zeros(N + 1, np.int64)
    np.cumsum(np.bincount(dst, minlength=N), out=starts[1:])
    edges_of = [eorder[starts[n]: starts[n + 1]] for n in range(N)]

    # host-side per-node score matrices M[d, n, h] and folded weights
    qp = q_nodes @ Wq + bq
    Mfull = np.empty((DIM, N, HEADS), np.float32)
    for h in range(HEADS):
        sl = slice(h * DH, (h + 1) * DH)
        Mfull[:, :, h] = (Wk[:, sl] * SCALE) @ qp[:, sl].T
    Mfull = Mfull.astype(np.float16)

    W2 = np.empty((DIM, 4 * DIM), np.float32)
    for h in range(HEADS):
        sl = slice(h * DH, (h + 1) * DH)
        W2[:, h * DIM:(h + 1) * DIM] = Wv[:, sl] @ Wo[sl, :]
    consts = {
        "W2": W2.astype(np.float16),
        "bo2": (bv @ Wo + bo).reshape(DIM, 1).astype(np.float32),
        "ID": np.eye(DIM, dtype=np.float16),
    }

    kT_ext = np.zeros((DIM, E + 1), np.float16)
    kT_ext[:, :E] = k_edges.T.astype(np.float16)
    v_ext = np.zeros((E + 1, DIM), np.float16)
    v_ext[:E] = v_edges.astype(np.float16)

    return plan, edges_of, consts, kT_ext, v_ext, Mfull, bo


def kernel(**inputs):
    from concourse.bass_utils import run_bass_kernel_spmd

    plan, edges_of, consts, kT_ext, v_ext, Mfull, bo = _prepare(inputs)

    nc = _build_module(plan)

    in_maps = []
    for c in range(NCORES):
        kvs = _pack_core_inputs(plan, c, kT_ext, v_ext, Mfull, edges_of)
        m = {"kvs": kvs}
        m.update(consts)
        in_maps.append(m)

    res = run_bass_kernel_spmd(nc, in_maps, core_ids=list(range(NCORES)))
    global LAST_RESULTS
    LAST_RESULTS = res

    out = np.zeros((N, DIM), np.float32)
    for c in range(NCORES):
        outT = res.results[c]["outT"]              # [128, SLOTP]
        cn = plan.core_nodes[c]
        gs = plan.rank2g[: len(cn)]
        out[np.array(cn, np.int64)] = outT[:, gs].T
    deg0 = plan.deg == 0
    if deg0.any():
        out[deg0] = bo
    return out


# revision 11
# speedup vs baseline: 2.2482x; 1.0209x over previous
"""Trainium2 Bass kernel for nn_NodeEdgeCrossAttention (v3).

Strategy (dst-sharded, zero-collective, fp16, minimal PE work):
  - Host sorts edges by destination node, greedily assigns nodes to 8 cores
    (balanced edge counts), and packs node edge-runs into 1024-column chunks
    with a slot pattern shared by all cores (SPMD: one program).  No per-node
    padding: slot boundaries are arbitrary; chunk tails are zero-padded.
  - Scores fold Wq/Wk/scale into per-node M matrices computed on host (O(N)):
    score[e,h] = M[dst_e,h] . k_raw_e.  bk cancels by softmax shift
    invariance; bv folds through Wo into the bias since sum(attn)==1.
  - Wv folds into Wo per head (W2_h = Wv[:,h] @ Wo[h,:]), so the device
    aggregates RAW v vectors; no per-edge v projection on device.
  - Per chunk on device: 1 fused DMA; NT score matmuls (k-tile
    stationary, M moving) -> [c=128, 4ns] PSUM per tile; 1 exp (scalar
    engine); 1 mask-multiply with the one-hot S (vector engine) -> exm;

    NT segment matmuls (exm stationary, [v|1] moving) accumulating
    [4ns, 129] in PSUM (weighted-v sums + softmax denominators);
    reciprocal of den + normalize multiply (vector) -> fp16; PE transpose
    into a persistent SBUF staging buffer.
  - Output blocks of 512 slots are projected with host-folded W2 + bias as
    soon as their slots are staged (overlapped with the chunk stream).
  - Numerics: fp16 linear tensors (fp8 fails the 2e-2 gate: attention-weight
    quantization error does not average down relative to the output), fp32
    accumulation, exp emits fp16 with a -ln16 bias that cancels in the
    normalize.
"""

import numpy as np

N, E, DIM, HEADS = 10000, 640000, 128, 4
DH = DIM // HEADS
NCORES = 8
CHUNK = 1024
NT = CHUNK // 128                 # k/v tiles per chunk
TILE = 128
SCALE = DH ** -0.5
NSMAX = 32                      # slots per chunk cap (PSUM: 4*NT*ns <= 1024 fp32)
LN16 = float(np.log(16.0))


class Plan:
    pass


def _make_plan(dst):
    """Greedy core assignment + shared chunk/slot pattern (no per-node pad)."""
    deg = np.bincount(dst, minlength=N)
    nz = np.where(deg > 0)[0]
    if deg.max() > CHUNK:
        raise NotImplementedError(f"max degree {deg.max()} > {CHUNK}")

    order = nz[np.argsort(-deg[nz], kind="stable")]
    loads = np.zeros(NCORES, np.int64)
    core_nodes = [[] for _ in range(NCORES)]
    for n in order:
        c = int(loads.argmin())
        core_nodes[c].append(int(n))
        loads[c] += deg[n]

    # Shared slot pattern: rank r -> max deg across cores at that rank.
    L = max(len(cn) for cn in core_nodes)
    pat = np.zeros(L, np.int64)
    for cn in core_nodes:
        d = deg[np.array(cn, np.int64)]
        pat[: len(d)] = np.maximum(pat[: len(d)], d)

    # First-fit decreasing bin packing of pattern slots into 512-col chunks.
    chunks = []                 # list of dict(slots=[(rank, col0, R)])
    open_rem = []               # remaining cols per open chunk
    for r in range(L):
        R = int(pat[r])
        placed = False
        for ci in range(len(chunks)):
            if open_rem[ci] >= R and len(chunks[ci]["slots"]) < NSMAX:
                col0 = CHUNK - open_rem[ci]
                chunks[ci]["slots"].append((r, col0, R))
                open_rem[ci] -= R
                placed = True
                break
        if not placed:
            chunks.append({"slots": [(r, 0, R)]})
            open_rem.append(CHUNK - R)

    # Global slot index g in (chunk, slot) order; map rank -> g.
    rank2g = np.full(L, -1, np.int64)
    g = 0
    for ch in chunks:
        ch["g0"] = g
        ch["ns"] = len(ch["slots"])
        for (r, _, _) in ch["slots"]:
            rank2g[r] = g
            g += 1

    p = Plan()
    p.deg = deg
    p.core_nodes = core_nodes
    p.chunks = chunks
    p.nchunks = len(chunks)
    p.L = L
    p.G = g                                   # total slots
    p.SLOTP = ((g + 511) // 512) * 512
    p.rank2g = rank2g
    # chunk fp16-element widths in the fused kvs tensor:
    # K(CHUNK) | V(NT*129) | S(NT*ns) | M(4ns)
    p.offs = []
    off = 0
    for ch in chunks:
        ns = ch["ns"]
        w = CHUNK + 129 * NT + (NT + 4) * ns
        p.offs.append(off)
        off += w
    p.KVW_total = off
    return p


def _pack_core_inputs(plan, c, kT_ext, v_ext, Mfull, edges_of):
    """Per-core fused kvs [128, KVW_total] fp16."""
    deg = plan.deg
    cn = plan.core_nodes[c]
    ncols = plan.nchunks * CHUNK
    idx = np.full(ncols, E, np.int64)          # E -> zero sentinel column
    for ch_i, ch in enumerate(plan.chunks):
        for (r, col0, R) in ch["slots"]:
            if r >= len(cn):
                continue
            node = cn[r]
            d = deg[node]
            g0 = ch_i * CHUNK + col0
            idx[g0: g0 + d] = edges_of[node]

    kT = kT_ext[:, idx]                        # [128, ncols] fp16
    vE = v_ext[idx]                            # [ncols, 128] fp16

    kvs = np.zeros((TILE, plan.KVW_total), np.float16)
    for ch_i, ch in enumerate(plan.chunks):
        ns = ch["ns"]
        off = plan.offs[ch_i]
        c0 = ch_i * CHUNK
        # K section [128 d, CHUNK c]
        kvs[:, off: off + CHUNK] = kT[:, c0: c0 + CHUNK]
        # V section: NT x [128 c, 128 d | 1]
        voff = off + CHUNK
        for t in range(NT):
            blk = kvs[:, voff + 129 * t: voff + 129 * t + 129]
            blk[:, 0:128] = vE[c0 + 128 * t: c0 + 128 * (t + 1)]
            blk[:, 128] = 1.0
        # S section: one-hot [128 c, (t, j)]
        soff = voff + 129 * NT
        for j, (r, col0, R) in enumerate(ch["slots"]):
            if r >= len(cn):
                continue
            d = int(deg[cn[r]])
            for t in range(NT):
                lo = max(col0, t * TILE)
                hi = min(col0 + d, (t + 1) * TILE)
                if lo < hi:
                    kvs[lo - t * TILE: hi - t * TILE, soff + t * ns + j] = 1.0
        # M section: [128 d, (j, h)]
        moff = soff + NT * ns
        for j, (r, col0, R) in enumerate(ch["slots"]):
            if r < len(cn):
                kvs[:, moff + 4 * j: moff + 4 * j + 4] = Mfull[:, cn[r], :]
    return kvs


# ---------------------------------------------------------------------------
# Device kernel emission
# ---------------------------------------------------------------------------

def _build_module(plan):
    import concourse.bacc as bacc
    import concourse.mybir as mybir
    import concourse.tile as tile
    from contextlib import ExitStack

    f16 = mybir.dt.float16
    f32 = mybir.dt.float32
    SLOTP = plan.SLOTP
    NBLK = SLOTP // 512

    nc = bacc.Bacc("TRN2", debug=False, num_devices=NCORES)

    kvs_d = nc.dram_tensor("kvs", [TILE, plan.KVW_total], f16,
                           kind="ExternalInput")
    W2_d = nc.dram_tensor("W2", [DIM, 4 * DIM], f16, kind="ExternalInput")
    bo2_d = nc.dram_tensor("bo2", [DIM, 1], f32, kind="ExternalInput")
    ID_d = nc.dram_tensor("ID", [DIM, DIM], f16, kind="ExternalInput")
    outT_d = nc.dram_tensor("outT", [DIM, SLOTP], f32, kind="ExternalOutput")

    Exp = mybir.ActivationFunctionType.Exp
    Ident = mybir.ActivationFunctionType.Identity
    mult = mybir.AluOpType.mult
    amax = mybir.AluOpType.max

    with ExitStack() as ctx:
        tc = ctx.enter_context(tile.TileContext(nc))
        cp = ctx.enter_context(tc.tile_pool(name="const", bufs=1))
        sp = ctx.enter_context(tc.tile_pool(name="persist", bufs=1))
        iop = ctx.enter_context(tc.tile_pool(name="io", bufs=8))
        xp = ctx.enter_context(tc.tile_pool(name="work", bufs=4))
        pp = ctx.enter_context(tc.tile_pool(name="ps", bufs=2, space="PSUM"))

        W2_sb = cp.tile([DIM, 4 * DIM], f16)
        nc.sync.dma_start(out=W2_sb[:], in_=W2_d[:, :])
        bo2_sb = cp.tile([DIM, 1], f32)
        nc.sync.dma_start(out=bo2_sb[:], in_=bo2_d[:, :])
        ln16_sb = cp.tile([DIM, 1], f32)
        nc.gpsimd.memset(ln16_sb[:], -LN16)
        ID_sb = cp.tile([DIM, DIM], f16)
        nc.sync.dma_start(out=ID_sb[:], in_=ID_d[:, :])
        stag = sp.tile([TILE, 4 * SLOTP], f16)
        stag_r = stag[:].rearrange("p (s h) -> p s h", h=4)

        def emit_block(b):
            out_ps = pp.tile([TILE, NT * TILE], f32, tag="score")
            for h in range(4):
                nc.tensor.matmul(
                    out=out_ps[:, 0:512],
                    lhsT=W2_sb[:, h * DIM:(h + 1) * DIM],
                    rhs=stag_r[:, b * 512:(b + 1) * 512, h],
                    start=(h == 0), stop=(h == 3))
            osb = xp.tile([DIM, 512], f32, tag="osb")
            nc.scalar.activation(out=osb[:], in_=out_ps[:, 0:512],
                                 func=Ident, bias=bo2_sb[:, 0:1])
            nc.sync.dma_start(out=outT_d[:, b * 512:(b + 1) * 512],
                              in_=osb[:])

        next_block = 0

        VOFF = CHUNK
        SOFF = CHUNK + 129 * NT

        kvts = {}

        def dma_chunk(j):
            if j >= plan.nchunks:
                return
            ch = plan.chunks[j]
            W = SOFF + (NT + 4) * ch["ns"]
            kvt = iop.tile([TILE, SOFF + (NT + 4) * NSMAX], f16, tag="kv")
            nc.sync.dma_start(out=kvt[:, 0:W],
                              in_=kvs_d[:, plan.offs[j]: plan.offs[j] + W])
            kvts[j] = kvt

        st = {}                  # per-chunk in-flight tiles
        PF = 3                   # DMA prefetch distance
        for j in range(PF):
            dma_chunk(j)

        # Software-pipelined steady state: per step i emit
        #   PE: scores_i | seg_{i-1} | transpose_{i-2}
        #   ACT: exp_i | stage-copy_{i-2}
        #   DVE: mask_i | rcp_{i-1} | normalize_{i-1}
        # so every strict-FIFO engine queue is ordered by data readiness.
        for i in range(plan.nchunks + 2):
            if i < plan.nchunks:
                ch = plan.chunks[i]
                ns = ch["ns"]
                kvt = kvts[i]
                ksec = kvt[:, 0:CHUNK]
                Msec = kvt[:, SOFF + NT * ns: SOFF + (NT + 4) * ns]
                score_ps = pp.tile([TILE, NT * TILE], f32, tag="score")
                for t in range(NT):
                    nc.tensor.matmul(
                        out=score_ps[:, t * TILE: t * TILE + 4 * ns],
                        lhsT=ksec[:, t * TILE:(t + 1) * TILE],
                        rhs=Msec[:],
                        start=True, stop=True)
                st[i] = {"ns": ns, "score": score_ps, "ch": ch}

            if i >= 1 and i - 1 < plan.nchunks:
                s = st[i - 1]
                ns = s["ns"]
                kvt = kvts[i - 1]
                vsec = kvt[:, VOFF:SOFF]
                park = pp.tile([TILE, 129], f32, tag="park")
                for t in range(NT):
                    nc.tensor.matmul(
                        out=park[0:4 * ns, :],
                        lhsT=s["exm"][:, t * TILE: t * TILE + 4 * ns],
                        rhs=vsec[:, 129 * t: 129 * t + 129],
                        start=(t == 0), stop=(t == NT - 1))
                s["park"] = park

            if i >= 2:
                s = st[i - 2]
                ns = s["ns"]
                tp_ps = pp.tile([TILE, TILE], f16, tag="tp")
                nc.tensor.transpose(out=tp_ps[0:TILE, 0:4 * ns],
                                    in_=s["aggN"][0:4 * ns, :],
                                    identity=ID_sb[0:4 * ns, 0:4 * ns])
                s["tp"] = tp_ps

            if i < plan.nchunks:
                s = st[i]
                ns = s["ns"]
                exs = xp.tile([TILE, NT * TILE], f16, tag="exs")
                score_r = s["score"][:].rearrange("p (t c) -> p t c", t=NT)
                exs_r = exs[:].rearrange("p (t c) -> p t c", t=NT)
                nc.scalar.activation(out=exs_r[:, :, 0:4 * ns],
                                     in_=score_r[:, :, 0:4 * ns],
                                     func=Exp, bias=ln16_sb[:, 0:1])
                s["exs"] = exs

            if i >= 2:
                s = st[i - 2]
                ns = s["ns"]
                g0 = s["ch"]["g0"]
                nc.scalar.copy(out=stag[:, 4 * g0: 4 * (g0 + ns)],
                               in_=s["tp"][0:TILE, 0:4 * ns])
                while (next_block < NBLK
                       and (g0 + ns) * 4 >= (next_block + 1) * 2048):
                    emit_block(next_block)
                    next_block += 1

            if i < plan.nchunks:
                s = st[i]
                ns = s["ns"]
                kvt = kvts[i]
                Ssec = kvt[:, SOFF:SOFF + NT * ns]
                exm = xp.tile([TILE, NT * TILE], f16, tag="exm")
                exm_r = exm[:].rearrange("p (t c) -> p t c", t=NT)
                exs_r = s["exs"][:].rearrange("p (t c) -> p t c", t=NT)
                nc.vector.tensor_tensor(
                    out=exm_r[:, :, 0:4 * ns].rearrange(
                        "p t (j h) -> p t j h", h=4),
                    in0=exs_r[:, :, 0:4 * ns].rearrange(
                        "p t (j h) -> p t j h", h=4),
                    in1=Ssec[:].rearrange("p (t j) -> p t j", t=NT)
                        .unsqueeze(-1).to_broadcast([TILE, NT, ns, 4]),
                    op=mult)
                s["exm"] = exm

            if i >= 1 and i - 1 < plan.nchunks:
                s = st[i - 1]
                ns = s["ns"]
                park = s["park"]
                rdent = xp.tile([TILE, 1], f32, tag="rden")
                nc.vector.reciprocal(out=rdent[0:4 * ns, 0:1],
                                     in_=park[0:4 * ns, 128:129])
                aggN = xp.tile([TILE, TILE], f16, tag="aggN")
                nc.vector.tensor_tensor(
                    out=aggN[0:4 * ns, :],
                    in0=park[0:4 * ns, 0:128],
                    in1=rdent[0:4 * ns, 0:1].to_broadcast([4 * ns, TILE]),
                    op=mult)
                s["aggN"] = aggN

            if i >= 2:
                st.pop(i - 2)
            dma_chunk(i + PF)

        while next_block < NBLK:
            emit_block(next_block)
            next_block += 1

    nc.compile()
    return nc


# ---------------------------------------------------------------------------
# Entry point
# ---------------------------------------------------------------------------

def _prepare(inputs):
    q_nodes = np.asarray(inputs["q_nodes"], np.float32)
    k_edges = np.asarray(inputs["k_edges"], np.float32)
    v_edges = np.asarray(inputs["v_edges"], np.float32)
    Wq = np.asarray(inputs["Wq"], np.float32)
    bq = np.asarray(inputs["bq"], np.float32)
    Wk = np.asarray(inputs["Wk"], np.float32)
    Wv = np.asarray(inputs["Wv"], np.float32)
    bv = np.asarray(inputs["bv"], np.float32)
    Wo = np.asarray(inputs["Wo"], np.float32)
    bo = np.asarray(inputs["bo"], np.float32)
    dst = np.asarray(inputs["edge_index"])[0].astype(np.int64)

    plan = _make_plan(dst)

    eorder = np.argsort(dst, kind="stable")
    starts = np.zeros(N + 1, np.int64)
    np.cumsum(np.bincount(dst, minlength=N), out=starts[1:])
    edges_of = [eorder[starts[n]: starts[n + 1]] for n in range(N)]

    # host-side per-node score matrices M[d, n, h] and folded weights
    qp = q_nodes @ Wq + bq
    Mfull = np.empty((DIM, N, HEADS), np.float32)
    for h in range(HEADS):
        sl = slice(h * DH, (h + 1) * DH)
        Mfull[:, :, h] = (Wk[:, sl] * SCALE) @ qp[:, sl].T
    Mfull = Mfull.astype(np.float16)

    W2 = np.empty((DIM, 4 * DIM), np.float32)
    for h in range(HEADS):
        sl = slice(h * DH, (h + 1) * DH)
        W2[:, h * DIM:(h + 1) * DIM] = Wv[:, sl] @ Wo[sl, :]
    consts = {
        "W2": W2.astype(np.float16),
        "bo2": (bv @ Wo + bo).reshape(DIM, 1).astype(np.float32),
        "ID": np.eye(DIM, dtype=np.float16),
    }

    kT_ext = np.zeros((DIM, E + 1), np.float16)
    kT_ext[:, :E] = k_edges.T.astype(np.float16)
    v_ext = np.zeros((E + 1, DIM), np.float16)
    v_ext[:E] = v_edges.astype(np.float16)

    return plan, edges_of, consts, kT_ext, v_ext, Mfull, bo


def kernel(**inputs):
    from concourse.bass_utils import run_bass_kernel_spmd

    plan, edges_of, consts, kT_ext, v_ext, Mfull, bo = _prepare(inputs)

    nc = _build_module(plan)

    in_maps = []
    for c in range(NCORES):
        kvs = _pack_core_inputs(plan, c, kT_ext, v_ext, Mfull, edges_of)
        m = {"kvs": kvs}
        m.update(consts)
        in_maps.append(m)

    res = run_bass_kernel_spmd(nc, in_maps, core_ids=list(range(NCORES)))
    global LAST_RESULTS
    LAST_RESULTS = res

    out = np.zeros((N, DIM), np.float32)
    for c in range(NCORES):
        outT = res.results[c]["outT"]              # [128, SLOTP]
        cn = plan.core_nodes[c]
        gs = plan.rank2g[: len(cn)]
        out[np.array(cn, np.int64)] = outT[:, gs].T
    deg0 = plan.deg == 0
    if deg0.any():
        out[deg0] = bo
    return out


# revision 12
# speedup vs baseline: 2.2517x; 1.0015x over previous
"""Trainium2 Bass kernel for nn_NodeEdgeCrossAttention (v3).

Strategy (dst-sharded, zero-collective, fp16, minimal PE work):
  - Host sorts edges by destination node, greedily assigns nodes to 8 cores
    (balanced edge counts), and packs node edge-runs into 1024-column chunks
    with a slot pattern shared by all cores (SPMD: one program).  No per-node
    padding: slot boundaries are arbitrary; chunk tails are zero-padded.
  - Scores fold Wq/Wk/scale into per-node M matrices computed on host (O(N)):
    score[e,h] = M[dst_e,h] . k_raw_e.  bk cancels by softmax shift
    invariance; bv folds through Wo into the bias since sum(attn)==1.
  - Wv folds into Wo per head (W2_h = Wv[:,h] @ Wo[h,:]), so the device
    aggregates RAW v vectors; no per-edge v projection on device.
  - Per chunk on device: 1 fused DMA; NT score matmuls (k-tile
    stationary, M moving) -> [c=128, 4ns] PSUM per tile; 1 exp (scalar
    engine); 1 mask-multiply with the one-hot S (vector engine) -> exm;

    NT segment matmuls (exm stationary, [v|1] moving) accumulating
    [4ns, 129] in PSUM (weighted-v sums + softmax denominators);
    reciprocal of den + normalize multiply (vector) -> fp16; PE transpose
    into a persistent SBUF staging buffer.
  - Output blocks of 512 slots are projected with host-folded W2 + bias as
    soon as their slots are staged (overlapped with the chunk stream).
  - Numerics: fp16 linear tensors (fp8 fails the 2e-2 gate: attention-weight
    quantization error does not average down relative to the output), fp32
    accumulation, exp emits fp16 with a -ln16 bias that cancels in the
    normalize.
"""

import numpy as np

N, E, DIM, HEADS = 10000, 640000, 128, 4
DH = DIM // HEADS
NCORES = 8
CHUNK = 1024
NT = CHUNK // 128                 # k/v tiles per chunk
TILE = 128
SCALE = DH ** -0.5
NSMAX = 32                      # slots per chunk cap (PSUM: 4*NT*ns <= 1024 fp32)
LN16 = float(np.log(16.0))


class Plan:
    pass


def _make_plan(dst):
    """Greedy core assignment + shared chunk/slot pattern (no per-node pad)."""
    deg = np.bincount(dst, minlength=N)
    nz = np.where(deg > 0)[0]
    if deg.max() > CHUNK:
        raise NotImplementedError(f"max degree {deg.max()} > {CHUNK}")

    order = nz[np.argsort(-deg[nz], kind="stable")]
    loads = np.zeros(NCORES, np.int64)
    core_nodes = [[] for _ in range(NCORES)]
    for n in order:
        c = int(loads.argmin())
        core_nodes[c].append(int(n))
        loads[c] += deg[n]

    # Shared slot pattern: rank r -> max deg across cores at that rank.
    L = max(len(cn) for cn in core_nodes)
    pat = np.zeros(L, np.int64)
    for cn in core_nodes:
        d = deg[np.array(cn, np.int64)]
        pat[: len(d)] = np.maximum(pat[: len(d)], d)

    # First-fit decreasing bin packing of pattern slots into 512-col chunks.
    chunks = []                 # list of dict(slots=[(rank, col0, R)])
    open_rem = []               # remaining cols per open chunk
    for r in range(L):
        R = int(pat[r])
        placed = False
        for ci in range(len(chunks)):
            if open_rem[ci] >= R and len(chunks[ci]["slots"]) < NSMAX:
                col0 = CHUNK - open_rem[ci]
                chunks[ci]["slots"].append((r, col0, R))
                open_rem[ci] -= R
                placed = True
                break
        if not placed:
            chunks.append({"slots": [(r, 0, R)]})
            open_rem.append(CHUNK - R)

    # Global slot index g in (chunk, slot) order; map rank -> g.
    rank2g = np.full(L, -1, np.int64)
    g = 0
    for ch in chunks:
        ch["g0"] = g
        ch["ns"] = len(ch["slots"])
        for (r, _, _) in ch["slots"]:
            rank2g[r] = g
            g += 1

    p = Plan()
    p.deg = deg
    p.core_nodes = core_nodes
    p.chunks = chunks
    p.nchunks = len(chunks)
    p.L = L
    p.G = g                                   # total slots
    p.SLOTP = ((g + 511) // 512) * 512
    p.rank2g = rank2g
    # chunk fp16-element widths in the fused kvs tensor:
    # K(CHUNK) | V(NT*129) | S(NT*ns) | M(4ns)
    p.offs = []
    off = 0
    for ch in chunks:
        ns = ch["ns"]
        w = CHUNK + 129 * NT + (NT + 4) * ns
        p.offs.append(off)
        off += w
    p.KVW_total = off
    return p


def _pack_core_inputs(plan, c, kT_ext, v_ext, Mfull, edges_of):
    """Per-core fused kvs [128, KVW_total] fp16."""
    deg = plan.deg
    cn = plan.core_nodes[c]
    ncols = plan.nchunks * CHUNK
    idx = np.full(ncols, E, np.int64)          # E -> zero sentinel column
    for ch_i, ch in enumerate(plan.chunks):
        for (r, col0, R) in ch["slots"]:
            if r >= len(cn):
                continue
            node = cn[r]
            d = deg[node]
            g0 = ch_i * CHUNK + col0
            idx[g0: g0 + d] = edges_of[node]

    kT = kT_ext[:, idx]                        # [128, ncols] fp16
    vE = v_ext[idx]                            # [ncols, 128] fp16

    kvs = np.zeros((TILE, plan.KVW_total), np.float16)
    for ch_i, ch in enumerate(plan.chunks):
        ns = ch["ns"]
        off = plan.offs[ch_i]
        c0 = ch_i * CHUNK
        # K section [128 d, CHUNK c]
        kvs[:, off: off + CHUNK] = kT[:, c0: c0 + CHUNK]
        # V section: NT x [128 c, 128 d | 1]
        voff = off + CHUNK
        for t in range(NT):
            blk = kvs[:, voff + 129 * t: voff + 129 * t + 129]
            blk[:, 0:128] = vE[c0 + 128 * t: c0 + 128 * (t + 1)]
            blk[:, 128] = 1.0
        # S section: one-hot [128 c, (t, j)]
        soff = voff + 129 * NT
        for j, (r, col0, R) in enumerate(ch["slots"]):
            if r >= len(cn):
                continue
            d = int(deg[cn[r]])
            for t in range(NT):
                lo = max(col0, t * TILE)
                hi = min(col0 + d, (t + 1) * TILE)
                if lo < hi:
                    kvs[lo - t * TILE: hi - t * TILE, soff + t * ns + j] = 1.0
        # M section: [128 d, (j, h)]
        moff = soff + NT * ns
        for j, (r, col0, R) in enumerate(ch["slots"]):
            if r < len(cn):
                kvs[:, moff + 4 * j: moff + 4 * j + 4] = Mfull[:, cn[r], :]
    return kvs


# ---------------------------------------------------------------------------
# Device kernel emission
# ---------------------------------------------------------------------------

def _build_module(plan):
    import concourse.bacc as bacc
    import concourse.mybir as mybir
    import concourse.tile as tile
    from contextlib import ExitStack

    f16 = mybir.dt.float16
    f32 = mybir.dt.float32
    SLOTP = plan.SLOTP
    NBLK = SLOTP // 512

    nc = bacc.Bacc("TRN2", debug=False, num_devices=NCORES)

    kvs_d = nc.dram_tensor("kvs", [TILE, plan.KVW_total], f16,
                           kind="ExternalInput")
    W2_d = nc.dram_tensor("W2", [DIM, 4 * DIM], f16, kind="ExternalInput")
    bo2_d = nc.dram_tensor("bo2", [DIM, 1], f32, kind="ExternalInput")
    ID_d = nc.dram_tensor("ID", [DIM, DIM], f16, kind="ExternalInput")
    outT_d = nc.dram_tensor("outT", [DIM, SLOTP], f32, kind="ExternalOutput")

    Exp = mybir.ActivationFunctionType.Exp
    Ident = mybir.ActivationFunctionType.Identity
    mult = mybir.AluOpType.mult
    amax = mybir.AluOpType.max

    with ExitStack() as ctx:
        tc = ctx.enter_context(tile.TileContext(nc))
        cp = ctx.enter_context(tc.tile_pool(name="const", bufs=1))
        sp = ctx.enter_context(tc.tile_pool(name="persist", bufs=1))
        iop = ctx.enter_context(tc.tile_pool(name="io", bufs=8))
        xp = ctx.enter_context(tc.tile_pool(name="work", bufs=4))
        pp = ctx.enter_context(tc.tile_pool(name="ps", bufs=2, space="PSUM"))

        W2_sb = cp.tile([DIM, 4 * DIM], f16)
        nc.sync.dma_start(out=W2_sb[:], in_=W2_d[:, :])
        bo2_sb = cp.tile([DIM, 1], f32)
        nc.sync.dma_start(out=bo2_sb[:], in_=bo2_d[:, :])
        ln16_sb = cp.tile([DIM, 1], f32)
        nc.gpsimd.memset(ln16_sb[:], -LN16)
        ID_sb = cp.tile([DIM, DIM], f16)
        nc.sync.dma_start(out=ID_sb[:], in_=ID_d[:, :])
        stag = sp.tile([TILE, 4 * SLOTP], f16)
        stag_r = stag[:].rearrange("p (s h) -> p s h", h=4)

        def emit_block(b):
            out_ps = pp.tile([TILE, NT * TILE], f32, tag="score")
            for h in range(4):
                nc.tensor.matmul(
                    out=out_ps[:, 0:512],
                    lhsT=W2_sb[:, h * DIM:(h + 1) * DIM],
                    rhs=stag_r[:, b * 512:(b + 1) * 512, h],
                    start=(h == 0), stop=(h == 3))
            osb = xp.tile([DIM, 512], f32, tag="osb")
            nc.scalar.activation(out=osb[:], in_=out_ps[:, 0:512],
                                 func=Ident, bias=bo2_sb[:, 0:1])
            nc.sync.dma_start(out=outT_d[:, b * 512:(b + 1) * 512],
                              in_=osb[:])

        next_block = 0

        VOFF = CHUNK
        SOFF = CHUNK + 129 * NT

        kvts = {}

        def dma_chunk(j):
            if j >= plan.nchunks:
                return
            ch = plan.chunks[j]
            W = SOFF + (NT + 4) * ch["ns"]
            kvt = iop.tile([TILE, SOFF + (NT + 4) * NSMAX], f16, tag="kv")
            nc.sync.dma_start(out=kvt[:, 0:W],
                              in_=kvs_d[:, plan.offs[j]: plan.offs[j] + W])
            kvts[j] = kvt

        st = {}                  # per-chunk in-flight tiles
        PF = 3                   # DMA prefetch distance
        for j in range(PF):
            dma_chunk(j)

        # Software-pipelined steady state with a 4-deep skew so that every
        # op is data-ready when its engine reaches it (strict-FIFO queues):
        #   PE:  scores_i | seg_{i-2} | transpose_{i-4}
        #   ACT: exp_i | stage-copy_{i-4}
        #   DVE: mask_{i-1} | rcp_{i-3} | normalize_{i-3}
        for i in range(plan.nchunks + 4):
            if i < plan.nchunks:
                ch = plan.chunks[i]
                ns = ch["ns"]
                kvt = kvts[i]
                ksec = kvt[:, 0:CHUNK]
                Msec = kvt[:, SOFF + NT * ns: SOFF + (NT + 4) * ns]
                score_ps = pp.tile([TILE, NT * TILE], f32, tag="score")
                for t in range(NT):
                    nc.tensor.matmul(
                        out=score_ps[:, t * TILE: t * TILE + 4 * ns],
                        lhsT=ksec[:, t * TILE:(t + 1) * TILE],
                        rhs=Msec[:],
                        start=True, stop=True)
                st[i] = {"ns": ns, "score": score_ps, "ch": ch}

            if 0 <= i - 2 < plan.nchunks:
                s = st[i - 2]
                ns = s["ns"]
                kvt = kvts[i - 2]
                vsec = kvt[:, VOFF:SOFF]
                park = pp.tile([TILE, 129], f32, tag="park")
                for t in range(NT):
                    nc.tensor.matmul(
                        out=park[0:4 * ns, :],
                        lhsT=s["exm"][:, t * TILE: t * TILE + 4 * ns],
                        rhs=vsec[:, 129 * t: 129 * t + 129],
                        start=(t == 0), stop=(t == NT - 1))
                s["park"] = park

            if 0 <= i - 4 < plan.nchunks:
                s = st[i - 4]
                ns = s["ns"]
                tp_ps = pp.tile([TILE, TILE], f16, tag="tp")
                nc.tensor.transpose(out=tp_ps[0:TILE, 0:4 * ns],
                                    in_=s["aggN"][0:4 * ns, :],
                                    identity=ID_sb[0:4 * ns, 0:4 * ns])
                s["tp"] = tp_ps

            if i < plan.nchunks:
                s = st[i]
                ns = s["ns"]
                exs = xp.tile([TILE, NT * TILE], f16, tag="exs")
                score_r = s["score"][:].rearrange("p (t c) -> p t c", t=NT)
                exs_r = exs[:].rearrange("p (t c) -> p t c", t=NT)
                nc.scalar.activation(out=exs_r[:, :, 0:4 * ns],
                                     in_=score_r[:, :, 0:4 * ns],
                                     func=Exp, bias=ln16_sb[:, 0:1])
                s["exs"] = exs

            if 0 <= i - 4 < plan.nchunks:
                s = st[i - 4]
                ns = s["ns"]
                g0 = s["ch"]["g0"]
                nc.scalar.copy(out=stag[:, 4 * g0: 4 * (g0 + ns)],
                               in_=s["tp"][0:TILE, 0:4 * ns])
                while (next_block < NBLK
                       and (g0 + ns) * 4 >= (next_block + 1) * 2048):
                    emit_block(next_block)
                    next_block += 1

            if 0 <= i - 1 < plan.nchunks:
                s = st[i - 1]
                ns = s["ns"]
                kvt = kvts[i - 1]
                Ssec = kvt[:, SOFF:SOFF + NT * ns]
                exm = xp.tile([TILE, NT * TILE], f16, tag="exm")
                exm_r = exm[:].rearrange("p (t c) -> p t c", t=NT)
                exs_r = s["exs"][:].rearrange("p (t c) -> p t c", t=NT)
                nc.vector.tensor_tensor(
                    out=exm_r[:, :, 0:4 * ns].rearrange(
                        "p t (j h) -> p t j h", h=4),
                    in0=exs_r[:, :, 0:4 * ns].rearrange(
                        "p t (j h) -> p t j h", h=4),
                    in1=Ssec[:].rearrange("p (t j) -> p t j", t=NT)
                        .unsqueeze(-1).to_broadcast([TILE, NT, ns, 4]),
                    op=mult)
                s["exm"] = exm

            if 0 <= i - 3 < plan.nchunks:
                s = st[i - 3]
                ns = s["ns"]
                park = s["park"]
                rdent = xp.tile([TILE, 1], f32, tag="rden")
                nc.vector.reciprocal(out=rdent[0:4 * ns, 0:1],
                                     in_=park[0:4 * ns, 128:129])
                aggN = xp.tile([TILE, TILE], f16, tag="aggN")
                nc.vector.tensor_tensor(
                    out=aggN[0:4 * ns, :],
                    in0=park[0:4 * ns, 0:128],
                    in1=rdent[0:4 * ns, 0:1].to_broadcast([4 * ns, TILE]),
                    op=mult)
                s["aggN"] = aggN

            if i >= 4:
                st.pop(i - 4, None)
            dma_chunk(i + PF)

        while next_block < NBLK:
            emit_block(next_block)
            next_block += 1

    nc.compile()
    return nc


# ---------------------------------------------------------------------------
# Entry point
# ---------------------------------------------------------------------------

def _prepare(inputs):
    q_nodes = np.asarray(inputs["q_nodes"], np.float32)
    k_edges = np.asarray(inputs["k_edges"], np.float32)
    v_edges = np.asarray(inputs["v_edges"], np.float32)
    Wq = np.asarray(inputs["Wq"], np.float32)
    bq = np.asarray(inputs["bq"], np.float32)
    Wk = np.asarray(inputs["Wk"], np.float32)
    Wv = np.asarray(inputs["Wv"], np.float32)
    bv = np.asarray(inputs["bv"], np.float32)
    Wo = np.asarray(inputs["Wo"], np.float32)
    bo = np.asarray(inputs["bo"], np.float32)
    dst = np.asarray(inputs["edge_index"])[0].astype(np.int64)

    plan = _make_plan(dst)

    eorder = np.argsort(dst, kind="stable")
    starts = np.zeros(N + 1, np.int64)
    np.cumsum(np.bincount(dst, minlength=N), out=starts[1:])
    edges_of = [eorder[starts[n]: starts[n + 1]] for n in range(N)]

    # host-side per-node score matrices M[d, n, h] and folded weights
    qp = q_nodes @ Wq + bq
    Mfull = np.empty((DIM, N, HEADS), np.float32)
    for h in range(HEADS):
        sl = slice(h * DH, (h + 1) * DH)
        Mfull[:, :, h] = (Wk[:, sl] * SCALE) @ qp[:, sl].T
    Mfull = Mfull.astype(np.float16)

    W2 = np.empty((DIM, 4 * DIM), np.float32)
    for h in range(HEADS):
        sl = slice(h * DH, (h + 1) * DH)
        W2[:, h * DIM:(h + 1) * DIM] = Wv[:, sl] @ Wo[sl, :]
    consts = {
        "W2": W2.astype(np.float16),
        "bo2": (bv @ Wo + bo).reshape(DIM, 1).astype(np.float32),
        "ID": np.eye(DIM, dtype=np.float16),
    }

    kT_ext = np.zeros((DIM, E + 1), np.float16)
    kT_ext[:, :E] = k_edges.T.astype(np.float16)
    v_ext = np.zeros((E + 1, DIM), np.float16)
    v_ext[:E] = v_edges.astype(np.float16)

    return plan, edges_of, consts, kT_ext, v_ext, Mfull, bo


def kernel(**inputs):
    from concourse.bass_utils import run_bass_kernel_spmd

    plan, edges_of, consts, kT_ext, v_ext, Mfull, bo = _prepare(inputs)

    nc = _build_module(plan)

    in_maps = []
    for c in range(NCORES):
        kvs = _pack_core_inputs(plan, c, kT_ext, v_ext, Mfull, edges_of)
        m = {"kvs": kvs}
        m.update(consts)
        in_maps.append(m)

    res = run_bass_kernel_spmd(nc, in_maps, core_ids=list(range(NCORES)))
    global LAST_RESULTS
    LAST_RESULTS = res

    out = np.zeros((N, DIM), np.float32)
    for c in range(NCORES):
        outT = res.results[c]["outT"]              # [128, SLOTP]
        cn = plan.core_nodes[c]
        gs = plan.rank2g[: len(cn)]
        out[np.array(cn, np.int64)] = outT[:, gs].T
    deg0 = plan.deg == 0
    if deg0.any():
        out[deg0] = bo
    return out


# revision 14
# speedup vs baseline: 2.2608x; 1.0041x over previous
"""Trainium2 Bass kernel for nn_NodeEdgeCrossAttention (v3).

Strategy (dst-sharded, zero-collective, fp16, minimal PE work):
  - Host sorts edges by destination node, greedily assigns nodes to 8 cores
    (balanced edge counts), and packs node edge-runs into 1024-column chunks
    with a slot pattern shared by all cores (SPMD: one program).  No per-node
    padding: slot boundaries are arbitrary; chunk tails are zero-padded.
  - Scores fold Wq/Wk/scale into per-node M matrices computed on host (O(N)):
    score[e,h] = M[dst_e,h] . k_raw_e.  bk cancels by softmax shift
    invariance; bv folds through Wo into the bias since sum(attn)==1.
  - Wv folds into Wo per head (W2_h = Wv[:,h] @ Wo[h,:]), so the device
    aggregates RAW v vectors; no per-edge v projection on device.
  - Per chunk on device: 1 fused DMA; NT score matmuls (k-tile
    stationary, M moving) -> [c=128, 4ns] PSUM per tile; 1 exp (scalar
    engine); 1 mask-multiply with the one-hot S (vector engine) -> exm;

    NT segment matmuls (exm stationary, [v|1] moving) accumulating
    [4ns, 129] in PSUM (weighted-v sums + softmax denominators);
    reciprocal of den + normalize multiply (vector) -> fp16; PE transpose
    into a persistent SBUF staging buffer.
  - Output blocks of 512 slots are projected with host-folded W2 + bias as
    soon as their slots are staged (overlapped with the chunk stream).
  - Numerics: fp16 linear tensors (fp8 fails the 2e-2 gate: attention-weight
    quantization error does not average down relative to the output), fp32
    accumulation, exp emits fp16 with a -ln16 bias that cancels in the
    normalize.
"""

import numpy as np

N, E, DIM, HEADS = 10000, 640000, 128, 4
DH = DIM // HEADS
NCORES = 8
CHUNK = 1024
NT = CHUNK // 128                 # k/v tiles per chunk
TILE = 128
SCALE = DH ** -0.5
NSMAX = 32                      # slots per chunk cap (PSUM: 4*NT*ns <= 1024 fp32)
LN16 = float(np.log(16.0))


class Plan:
    pass


def _make_plan(dst):
    """Greedy core assignment + shared chunk/slot pattern (no per-node pad)."""
    deg = np.bincount(dst, minlength=N)
    nz = np.where(deg > 0)[0]
    if deg.max() > CHUNK:
        raise NotImplementedError(f"max degree {deg.max()} > {CHUNK}")

    order = nz[np.argsort(-deg[nz], kind="stable")]
    loads = np.zeros(NCORES, np.int64)
    core_nodes = [[] for _ in range(NCORES)]
    for n in order:
        c = int(loads.argmin())
        core_nodes[c].append(int(n))
        loads[c] += deg[n]

    # Shared slot pattern: rank r -> max deg across cores at that rank.
    L = max(len(cn) for cn in core_nodes)
    pat = np.zeros(L, np.int64)
    for cn in core_nodes:
        d = deg[np.array(cn, np.int64)]
        pat[: len(d)] = np.maximum(pat[: len(d)], d)

    # First-fit decreasing bin packing of pattern slots into 512-col chunks.
    chunks = []                 # list of dict(slots=[(rank, col0, R)])
    open_rem = []               # remaining cols per open chunk
    for r in range(L):
        R = int(pat[r])
        placed = False
        for ci in range(len(chunks)):
            if open_rem[ci] >= R and len(chunks[ci]["slots"]) < NSMAX:
                col0 = CHUNK - open_rem[ci]
                chunks[ci]["slots"].append((r, col0, R))
                open_rem[ci] -= R
                placed = True
                break
        if not placed:
            chunks.append({"slots": [(r, 0, R)]})
            open_rem.append(CHUNK - R)

    # Global slot index g in (chunk, slot) order; map rank -> g.
    rank2g = np.full(L, -1, np.int64)
    g = 0
    for ch in chunks:
        ch["g0"] = g
        ch["ns"] = len(ch["slots"])
        for (r, _, _) in ch["slots"]:
            rank2g[r] = g
            g += 1

    p = Plan()
    p.deg = deg
    p.core_nodes = core_nodes
    p.chunks = chunks
    p.nchunks = len(chunks)
    p.L = L
    p.G = g                                   # total slots
    p.SLOTP = ((g + 511) // 512) * 512
    p.rank2g = rank2g
    # chunk fp16-element widths in the fused kvs tensor:
    # K(CHUNK) | V(NT*129) | S(NT*ns) | M(4ns)
    p.offs = []
    off = 0
    for ch in chunks:
        ns = ch["ns"]
        w = CHUNK + 129 * NT + (NT + 4) * ns
        p.offs.append(off)
        off += w
    p.KVW_total = off
    return p


def _pack_core_inputs(plan, c, kT_ext, v_ext, Mfull, edges_of):
    """Per-core fused kvs [128, KVW_total] fp16."""
    deg = plan.deg
    cn = plan.core_nodes[c]
    ncols = plan.nchunks * CHUNK
    idx = np.full(ncols, E, np.int64)          # E -> zero sentinel column
    for ch_i, ch in enumerate(plan.chunks):
        for (r, col0, R) in ch["slots"]:
            if r >= len(cn):
                continue
            node = cn[r]
            d = deg[node]
            g0 = ch_i * CHUNK + col0
            idx[g0: g0 + d] = edges_of[node]

    kT = kT_ext[:, idx]                        # [128, ncols] fp16
    vE = v_ext[idx]                            # [ncols, 128] fp16

    kvs = np.zeros((TILE, plan.KVW_total), np.float16)
    for ch_i, ch in enumerate(plan.chunks):
        ns = ch["ns"]
        off = plan.offs[ch_i]
        c0 = ch_i * CHUNK
        # K section [128 d, CHUNK c]
        kvs[:, off: off + CHUNK] = kT[:, c0: c0 + CHUNK]
        # V section: NT x [128 c, 128 d | 1]
        voff = off + CHUNK
        for t in range(NT):
            blk = kvs[:, voff + 129 * t: voff + 129 * t + 129]
            blk[:, 0:128] = vE[c0 + 128 * t: c0 + 128 * (t + 1)]
            blk[:, 128] = 1.0
        # S section: one-hot [128 c, (t, j)]
        soff = voff + 129 * NT
        for j, (r, col0, R) in enumerate(ch["slots"]):
            if r >= len(cn):
                continue
            d = int(deg[cn[r]])
            for t in range(NT):
                lo = max(col0, t * TILE)
                hi = min(col0 + d, (t + 1) * TILE)
                if lo < hi:
                    kvs[lo - t * TILE: hi - t * TILE, soff + t * ns + j] = 1.0
        # M section: [128 d, (j, h)]
        moff = soff + NT * ns
        for j, (r, col0, R) in enumerate(ch["slots"]):
            if r < len(cn):
                kvs[:, moff + 4 * j: moff + 4 * j + 4] = Mfull[:, cn[r], :]
    return kvs


# ---------------------------------------------------------------------------
# Device kernel emission
# ---------------------------------------------------------------------------

def _build_module(plan):
    import concourse.bacc as bacc
    import concourse.mybir as mybir
    import concourse.tile as tile
    from contextlib import ExitStack

    f16 = mybir.dt.float16
    f32 = mybir.dt.float32
    SLOTP = plan.SLOTP
    NBLK = SLOTP // 512

    nc = bacc.Bacc("TRN2", debug=False, num_devices=NCORES)

    kvs_d = nc.dram_tensor("kvs", [TILE, plan.KVW_total], f16,
                           kind="ExternalInput")
    W2_d = nc.dram_tensor("W2", [DIM, 4 * DIM], f16, kind="ExternalInput")
    bo2_d = nc.dram_tensor("bo2", [DIM, 1], f32, kind="ExternalInput")
    ID_d = nc.dram_tensor("ID", [DIM, DIM], f16, kind="ExternalInput")
    outT_d = nc.dram_tensor("outT", [DIM, SLOTP], f32, kind="ExternalOutput")

    Exp = mybir.ActivationFunctionType.Exp
    Ident = mybir.ActivationFunctionType.Identity
    mult = mybir.AluOpType.mult
    amax = mybir.AluOpType.max

    with ExitStack() as ctx:
        tc = ctx.enter_context(tile.TileContext(nc))
        cp = ctx.enter_context(tc.tile_pool(name="const", bufs=1))
        sp = ctx.enter_context(tc.tile_pool(name="persist", bufs=1))
        iop = ctx.enter_context(tc.tile_pool(name="io", bufs=8))
        xp = ctx.enter_context(tc.tile_pool(name="work", bufs=4))
        pp = ctx.enter_context(tc.tile_pool(name="ps", bufs=2, space="PSUM"))

        W2_sb = cp.tile([DIM, 4 * DIM], f16)
        nc.sync.dma_start(out=W2_sb[:], in_=W2_d[:, :])
        bo2_sb = cp.tile([DIM, 1], f32)
        nc.sync.dma_start(out=bo2_sb[:], in_=bo2_d[:, :])
        ln16_sb = cp.tile([DIM, 1], f32)
        nc.gpsimd.memset(ln16_sb[:], -LN16)
        ID_sb = cp.tile([DIM, DIM], f16)
        nc.sync.dma_start(out=ID_sb[:], in_=ID_d[:, :])
        stag = sp.tile([TILE, 4 * SLOTP], f16)
        stag_r = stag[:].rearrange("p (s h) -> p s h", h=4)

        def emit_block(b):
            out_ps = pp.tile([TILE, NT * TILE], f32, tag="score")
            for h in range(4):
                nc.tensor.matmul(
                    out=out_ps[:, 0:512],
                    lhsT=W2_sb[:, h * DIM:(h + 1) * DIM],
                    rhs=stag_r[:, b * 512:(b + 1) * 512, h],
                    start=(h == 0), stop=(h == 3))
            osb = xp.tile([DIM, 512], f32, tag="osb")
            nc.scalar.activation(out=osb[:], in_=out_ps[:, 0:512],
                                 func=Ident, bias=bo2_sb[:, 0:1])
            nc.sync.dma_start(out=outT_d[:, b * 512:(b + 1) * 512],
                              in_=osb[:])

        next_block = 0

        VOFF = CHUNK
        SOFF = CHUNK + 129 * NT

        kvts = {}

        def dma_chunk(j):
            if j >= plan.nchunks:
                return
            ch = plan.chunks[j]
            W = SOFF + (NT + 4) * ch["ns"]
            kvt = iop.tile([TILE, SOFF + (NT + 4) * NSMAX], f16, tag="kv")
            nc.sync.dma_start(out=kvt[:, 0:W],
                              in_=kvs_d[:, plan.offs[j]: plan.offs[j] + W])
            kvts[j] = kvt

        st = {}                  # per-chunk in-flight tiles
        PF = 3                   # DMA prefetch distance
        for j in range(PF):
            dma_chunk(j)

        # HAM warmup: ~4.5us of back-to-back dummy matmuls so the PE clock
        # gate opens (1.2 -> 2.4 GHz).  Steady-state gaps are far below the
        # ~3.4us idle window needed to re-throttle, so it stays warm.
        warm_ps = pp.tile([TILE, NT * TILE], f32, tag="score")
        for _ in range(44):
            nc.tensor.matmul(out=warm_ps[:, 0:TILE],
                             lhsT=W2_sb[:, 0:DIM],
                             rhs=W2_sb[:, 0:DIM],
                             start=True, stop=True)

        # Software-pipelined steady state with a 4-deep skew so that every
        # op is data-ready when its engine reaches it (strict-FIFO queues):
        #   PE:  scores_i | seg_{i-2} | transpose_{i-4}
        #   ACT: exp_i | stage-copy_{i-4}
        #   DVE: mask_{i-1} | rcp_{i-3} | normalize_{i-3}
        for i in range(plan.nchunks + 4):
            if i < plan.nchunks:
                ch = plan.chunks[i]
                ns = ch["ns"]
                kvt = kvts[i]
                ksec = kvt[:, 0:CHUNK]
                Msec = kvt[:, SOFF + NT * ns: SOFF + (NT + 4) * ns]
                score_ps = pp.tile([TILE, NT * TILE], f32, tag="score")
                for t in range(NT):
                    nc.tensor.matmul(
                        out=score_ps[:, t * TILE: t * TILE + 4 * ns],
                        lhsT=ksec[:, t * TILE:(t + 1) * TILE],
                        rhs=Msec[:],
                        start=True, stop=True)
                st[i] = {"ns": ns, "score": score_ps, "ch": ch}

            if 0 <= i - 2 < plan.nchunks:
                s = st[i - 2]
                ns = s["ns"]
                kvt = kvts[i - 2]
                vsec = kvt[:, VOFF:SOFF]
                park = pp.tile([TILE, 129], f32, tag="park")
                for t in range(NT):
                    nc.tensor.matmul(
                        out=park[0:4 * ns, :],
                        lhsT=s["exm"][:, t * TILE: t * TILE + 4 * ns],
                        rhs=vsec[:, 129 * t: 129 * t + 129],
                        start=(t == 0), stop=(t == NT - 1))
                s["park"] = park

            if 0 <= i - 4 < plan.nchunks:
                s = st[i - 4]
                ns = s["ns"]
                tp_ps = pp.tile([TILE, TILE], f16, tag="tp")
                nc.tensor.transpose(out=tp_ps[0:TILE, 0:4 * ns],
                                    in_=s["aggN"][0:4 * ns, :],
                                    identity=ID_sb[0:4 * ns, 0:4 * ns])
                s["tp"] = tp_ps

            if i < plan.nchunks:
                s = st[i]
                ns = s["ns"]
                exs = xp.tile([TILE, NT * TILE], f16, tag="exs")
                score_r = s["score"][:].rearrange("p (t c) -> p t c", t=NT)
                exs_r = exs[:].rearrange("p (t c) -> p t c", t=NT)
                nc.scalar.activation(out=exs_r[:, :, 0:4 * ns],
                                     in_=score_r[:, :, 0:4 * ns],
                                     func=Exp, bias=ln16_sb[:, 0:1])
                s["exs"] = exs

            if 0 <= i - 4 < plan.nchunks:
                s = st[i - 4]
                ns = s["ns"]
                g0 = s["ch"]["g0"]
                nc.scalar.copy(out=stag[:, 4 * g0: 4 * (g0 + ns)],
                               in_=s["tp"][0:TILE, 0:4 * ns])
                while (next_block < NBLK
                       and (g0 + ns) * 4 >= (next_block + 1) * 2048):
                    emit_block(next_block)
                    next_block += 1

            if 0 <= i - 1 < plan.nchunks:
                s = st[i - 1]
                ns = s["ns"]
                kvt = kvts[i - 1]
                Ssec = kvt[:, SOFF:SOFF + NT * ns]
                exm = xp.tile([TILE, NT * TILE], f16, tag="exm")
                exm_r = exm[:].rearrange("p (t c) -> p t c", t=NT)
                exs_r = s["exs"][:].rearrange("p (t c) -> p t c", t=NT)
                nc.vector.tensor_tensor(
                    out=exm_r[:, :, 0:4 * ns].rearrange(
                        "p t (j h) -> p t j h", h=4),
                    in0=exs_r[:, :, 0:4 * ns].rearrange(
                        "p t (j h) -> p t j h", h=4),
                    in1=Ssec[:].rearrange("p (t j) -> p t j", t=NT)
                        .unsqueeze(-1).to_broadcast([TILE, NT, ns, 4]),
                    op=mult)
                s["exm"] = exm

            if 0 <= i - 3 < plan.nchunks:
                s = st[i - 3]
                ns = s["ns"]
                park = s["park"]
                rdent = xp.tile([TILE, 1], f32, tag="rden")
                nc.vector.reciprocal(out=rdent[0:4 * ns, 0:1],
                                     in_=park[0:4 * ns, 128:129])
                aggN = xp.tile([TILE, TILE], f16, tag="aggN")
                nc.vector.tensor_tensor(
                    out=aggN[0:4 * ns, :],
                    in0=park[0:4 * ns, 0:128],
                    in1=rdent[0:4 * ns, 0:1].to_broadcast([4 * ns, TILE]),
                    op=mult)
                s["aggN"] = aggN

            if i >= 4:
                st.pop(i - 4, None)
            dma_chunk(i + PF)

        while next_block < NBLK:
            emit_block(next_block)
            next_block += 1

    nc.compile()
    return nc


# ---------------------------------------------------------------------------
# Entry point
# ---------------------------------------------------------------------------

def _prepare(inputs):
    q_nodes = np.asarray(inputs["q_nodes"], np.float32)
    k_edges = np.asarray(inputs["k_edges"], np.float32)
    v_edges = np.asarray(inputs["v_edges"], np.float32)
    Wq = np.asarray(inputs["Wq"], np.float32)
    bq = np.asarray(inputs["bq"], np.float32)
    Wk = np.asarray(inputs["Wk"], np.float32)
    Wv = np.asarray(inputs["Wv"], np.float32)
    bv = np.asarray(inputs["bv"], np.float32)
    Wo = np.asarray(inputs["Wo"], np.float32)
    bo = np.asarray(inputs["bo"], np.float32)
    dst = np.asarray(inputs["edge_index"])[0].astype(np.int64)

    plan = _make_plan(dst)

    eorder = np.argsort(dst, kind="stable")
    starts = np.zeros(N + 1, np.int64)
    np.cumsum(np.bincount(dst, minlength=N), out=starts[1:])
    edges_of = [eorder[starts[n]: starts[n + 1]] for n in range(N)]

    # host-side per-node score matrices M[d, n, h] and folded weights
    qp = q_nodes @ Wq + bq
    Mfull = np.empty((DIM, N, HEADS), np.float32)
    for h in range(HEADS):
        sl = slice(h * DH, (h + 1) * DH)
        Mfull[:, :, h] = (Wk[:, sl] * SCALE) @ qp[:, sl].T
    Mfull = Mfull.astype(np.float16)

    W2 = np.empty((DIM, 4 * DIM), np.float32)
    for h in range(HEADS):
        sl = slice(h * DH, (h + 1) * DH)
        W2[:, h * DIM:(h + 1) * DIM] = Wv[:, sl] @ Wo[sl, :]
    consts = {
        "W2": W2.astype(np.float16),
        "bo2": (bv @ Wo + bo).reshape(DIM, 1).astype(np.float32),
        "ID": np.eye(DIM, dtype=np.float16),
    }

    kT_ext = np.zeros((DIM, E + 1), np.float16)
    kT_ext[:, :E] = k_edges.T.astype(np.float16)
    v_ext = np.zeros((E + 1, DIM), np.float16)
    v_ext[:E] = v_edges.astype(np.float16)

    return plan, edges_of, consts, kT_ext, v_ext, Mfull, bo


def kernel(**inputs):
    from concourse.bass_utils import run_bass_kernel_spmd

    plan, edges_of, consts, kT_ext, v_ext, Mfull, bo = _prepare(inputs)

    nc = _build_module(plan)

    in_maps = []
    for c in range(NCORES):
        kvs = _pack_core_inputs(plan, c, kT_ext, v_ext, Mfull, edges_of)
        m = {"kvs": kvs}
        m.update(consts)
        in_maps.append(m)

    res = run_bass_kernel_spmd(nc, in_maps, core_ids=list(range(NCORES)))
    global LAST_RESULTS
    LAST_RESULTS = res

    out = np.zeros((N, DIM), np.float32)
    for c in range(NCORES):
        outT = res.results[c]["outT"]              # [128, SLOTP]
        cn = plan.core_nodes[c]
        gs = plan.rank2g[: len(cn)]
        out[np.array(cn, np.int64)] = outT[:, gs].T
    deg0 = plan.deg == 0
    if deg0.any():
        out[deg0] = bo
    return out


# revision 16
# speedup vs baseline: 2.2965x; 1.0158x over previous
"""Trainium2 Bass kernel for nn_NodeEdgeCrossAttention (v3).

Strategy (dst-sharded, zero-collective, fp16, minimal PE work):
  - Host sorts edges by destination node, greedily assigns nodes to 8 cores
    (balanced edge counts), and packs node edge-runs into 1024-column chunks
    with a slot pattern shared by all cores (SPMD: one program).  No per-node
    padding: slot boundaries are arbitrary; chunk tails are zero-padded.
  - Scores fold Wq/Wk/scale into per-node M matrices computed on host (O(N)):
    score[e,h] = M[dst_e,h] . k_raw_e.  bk cancels by softmax shift
    invariance; bv folds through Wo into the bias since sum(attn)==1.
  - Wv folds into Wo per head (W2_h = Wv[:,h] @ Wo[h,:]), so the device
    aggregates RAW v vectors; no per-edge v projection on device.
  - Per chunk on device: 1 fused DMA; NT score matmuls (k-tile
    stationary, M moving) -> [c=128, 4ns] PSUM per tile; 1 exp (scalar
    engine); 1 mask-multiply with the one-hot S (vector engine) -> exm;

    NT segment matmuls (exm stationary, [v|1] moving) accumulating
    [4ns, 129] in PSUM (weighted-v sums + softmax denominators);
    reciprocal of den + normalize multiply (vector) -> fp16; PE transpose
    into a persistent SBUF staging buffer.
  - Output blocks of 512 slots are projected with host-folded W2 + bias as
    soon as their slots are staged (overlapped with the chunk stream).
  - Numerics: fp16 linear tensors (fp8 fails the 2e-2 gate: attention-weight
    quantization error does not average down relative to the output), fp32
    accumulation, exp emits fp16 with a -ln16 bias that cancels in the
    normalize.
"""

import numpy as np

N, E, DIM, HEADS = 10000, 640000, 128, 4
DH = DIM // HEADS
NCORES = 8
CHUNK = 1024
NT = CHUNK // 128                 # k/v tiles per chunk
TILE = 128
SCALE = DH ** -0.5
NSMAX = 32                      # slots per chunk cap (PSUM: 4*NT*ns <= 1024 fp32)
LN16 = float(np.log(16.0))


class Plan:
    pass


def _make_plan(dst):
    """Greedy core assignment + shared chunk/slot pattern (no per-node pad)."""
    deg = np.bincount(dst, minlength=N)
    nz = np.where(deg > 0)[0]
    if deg.max() > CHUNK:
        raise NotImplementedError(f"max degree {deg.max()} > {CHUNK}")

    order = nz[np.argsort(-deg[nz], kind="stable")]
    loads = np.zeros(NCORES, np.int64)
    core_nodes = [[] for _ in range(NCORES)]
    for n in order:
        c = int(loads.argmin())
        core_nodes[c].append(int(n))
        loads[c] += deg[n]

    # Shared slot pattern: rank r -> max deg across cores at that rank.
    L = max(len(cn) for cn in core_nodes)
    pat = np.zeros(L, np.int64)
    for cn in core_nodes:
        d = deg[np.array(cn, np.int64)]
        pat[: len(d)] = np.maximum(pat[: len(d)], d)

    # First-fit decreasing bin packing of pattern slots into 512-col chunks.
    chunks = []                 # list of dict(slots=[(rank, col0, R)])
    open_rem = []               # remaining cols per open chunk
    for r in range(L):
        R = int(pat[r])
        placed = False
        for ci in range(len(chunks)):
            if open_rem[ci] >= R and len(chunks[ci]["slots"]) < NSMAX:
                col0 = CHUNK - open_rem[ci]
                chunks[ci]["slots"].append((r, col0, R))
                open_rem[ci] -= R
                placed = True
                break
        if not placed:
            chunks.append({"slots": [(r, 0, R)]})
            open_rem.append(CHUNK - R)

    # Global slot index g in (chunk, slot) order; map rank -> g.
    rank2g = np.full(L, -1, np.int64)
    g = 0
    for ch in chunks:
        ch["g0"] = g
        ch["ns"] = len(ch["slots"])
        for (r, _, _) in ch["slots"]:
            rank2g[r] = g
            g += 1

    p = Plan()
    p.deg = deg
    p.core_nodes = core_nodes
    p.chunks = chunks
    p.nchunks = len(chunks)
    p.L = L
    p.G = g                                   # total slots
    p.SLOTP = ((g + 511) // 512) * 512
    p.rank2g = rank2g
    # chunk fp16-element widths in the fused kvs tensor:
    # K(CHUNK) | V(NT*129) | S(NT*ns) | M(4ns)
    p.offs = []
    off = 0
    for ch in chunks:
        ns = ch["ns"]
        w = CHUNK + 129 * NT + (NT + 4) * ns
        p.offs.append(off)
        off += w
    p.KVW_total = off
    return p


def _pack_core_inputs(plan, c, kT_ext, v_ext, Mfull, edges_of):
    """Per-core fused kvs [128, KVW_total] fp16."""
    deg = plan.deg
    cn = plan.core_nodes[c]
    ncols = plan.nchunks * CHUNK
    idx = np.full(ncols, E, np.int64)          # E -> zero sentinel column
    for ch_i, ch in enumerate(plan.chunks):
        for (r, col0, R) in ch["slots"]:
            if r >= len(cn):
                continue
            node = cn[r]
            d = deg[node]
            g0 = ch_i * CHUNK + col0
            idx[g0: g0 + d] = edges_of[node]

    kT = kT_ext[:, idx]                        # [128, ncols] fp16
    vE = v_ext[idx]                            # [ncols, 128] fp16

    kvs = np.zeros((TILE, plan.KVW_total), np.float16)
    for ch_i, ch in enumerate(plan.chunks):
        ns = ch["ns"]
        off = plan.offs[ch_i]
        c0 = ch_i * CHUNK
        # K section [128 d, CHUNK c]
        kvs[:, off: off + CHUNK] = kT[:, c0: c0 + CHUNK]
        # V section: NT x [128 c, 128 d | 1]
        voff = off + CHUNK
        for t in range(NT):
            blk = kvs[:, voff + 129 * t: voff + 129 * t + 129]
            blk[:, 0:128] = vE[c0 + 128 * t: c0 + 128 * (t + 1)]
            blk[:, 128] = 1.0
        # S section: one-hot [128 c, (t, j)]
        soff = voff + 129 * NT
        for j, (r, col0, R) in enumerate(ch["slots"]):
            if r >= len(cn):
                continue
            d = int(deg[cn[r]])
            for t in range(NT):
                lo = max(col0, t * TILE)
                hi = min(col0 + d, (t + 1) * TILE)
                if lo < hi:
                    kvs[lo - t * TILE: hi - t * TILE, soff + t * ns + j] = 1.0
        # M section: [128 d, (j, h)]
        moff = soff + NT * ns
        for j, (r, col0, R) in enumerate(ch["slots"]):
            if r < len(cn):
                kvs[:, moff + 4 * j: moff + 4 * j + 4] = Mfull[:, cn[r], :]
    return kvs


# ---------------------------------------------------------------------------
# Device kernel emission
# ---------------------------------------------------------------------------

def _build_module(plan):
    import concourse.bacc as bacc
    import concourse.mybir as mybir
    import concourse.tile as tile
    from contextlib import ExitStack

    f16 = mybir.dt.float16
    f32 = mybir.dt.float32
    SLOTP = plan.SLOTP
    NBLK = SLOTP // 512

    nc = bacc.Bacc("TRN2", debug=False, num_devices=NCORES)

    kvs_d = nc.dram_tensor("kvs", [TILE, plan.KVW_total], f16,
                           kind="ExternalInput")
    W2_d = nc.dram_tensor("W2", [DIM, 4 * DIM], f16, kind="ExternalInput")
    bo2_d = nc.dram_tensor("bo2", [DIM, 1], f32, kind="ExternalInput")
    ID_d = nc.dram_tensor("ID", [DIM, DIM], f16, kind="ExternalInput")
    outT_d = nc.dram_tensor("outT", [DIM, SLOTP], f32, kind="ExternalOutput")

    Exp = mybir.ActivationFunctionType.Exp
    Ident = mybir.ActivationFunctionType.Identity
    mult = mybir.AluOpType.mult
    amax = mybir.AluOpType.max

    with ExitStack() as ctx:
        tc = ctx.enter_context(tile.TileContext(nc))
        cp = ctx.enter_context(tc.tile_pool(name="const", bufs=1))
        sp = ctx.enter_context(tc.tile_pool(name="persist", bufs=1))
        iop = ctx.enter_context(tc.tile_pool(name="io", bufs=8))
        xp = ctx.enter_context(tc.tile_pool(name="work", bufs=4))
        pp = ctx.enter_context(tc.tile_pool(name="ps", bufs=2, space="PSUM"))
        pp1 = ctx.enter_context(tc.tile_pool(name="ps1", bufs=1, space="PSUM"))
        dp = ctx.enter_context(tc.tile_pool(name="dummy", bufs=1, space="PSUM"))

        W2_sb = cp.tile([DIM, 4 * DIM], f16)
        nc.sync.dma_start(out=W2_sb[:], in_=W2_d[:, :])
        bo2_sb = cp.tile([DIM, 1], f32)
        nc.sync.dma_start(out=bo2_sb[:], in_=bo2_d[:, :])
        ln16_sb = cp.tile([DIM, 1], f32)
        nc.gpsimd.memset(ln16_sb[:], -LN16)
        ID_sb = cp.tile([DIM, DIM], f16)
        nc.sync.dma_start(out=ID_sb[:], in_=ID_d[:, :])
        stag = sp.tile([TILE, 4 * SLOTP], f16)
        stag_r = stag[:].rearrange("p (s h) -> p s h", h=4)

        def emit_block(b):
            out_ps = pp.tile([TILE, NT * TILE], f32, tag="score")
            for h in range(4):
                nc.tensor.matmul(
                    out=out_ps[:, 0:512],
                    lhsT=W2_sb[:, h * DIM:(h + 1) * DIM],
                    rhs=stag_r[:, b * 512:(b + 1) * 512, h],
                    start=(h == 0), stop=(h == 3))
            osb = xp.tile([DIM, 512], f32, tag="osb")
            nc.scalar.activation(out=osb[:], in_=out_ps[:, 0:512],
                                 func=Ident, bias=bo2_sb[:, 0:1])
            nc.sync.dma_start(out=outT_d[:, b * 512:(b + 1) * 512],
                              in_=osb[:])

        next_block = 0

        VOFF = CHUNK
        SOFF = CHUNK + 129 * NT

        kvts = {}

        def dma_chunk(j):
            if j >= plan.nchunks:
                return
            ch = plan.chunks[j]
            W = SOFF + (NT + 4) * ch["ns"]
            kvt = iop.tile([TILE, SOFF + (NT + 4) * NSMAX], f16, tag="kv")
            nc.sync.dma_start(out=kvt[:, 0:W],
                              in_=kvs_d[:, plan.offs[j]: plan.offs[j] + W])
            kvts[j] = kvt

        st = {}                  # per-chunk in-flight tiles
        PF = 3                   # DMA prefetch distance
        for j in range(PF):
            dma_chunk(j)

        # HAM warmup: ~4.5us of back-to-back dummy matmuls so the PE clock
        # gate opens (1.2 -> 2.4 GHz).  Steady-state gaps are far below the
        # ~3.4us idle window needed to re-throttle, so it stays warm.
        warm_ps = dp.tile([1, TILE], f32, tag="warm")

        def keep_warm(n, cols=TILE):
            for _ in range(n):
                nc.tensor.matmul(out=warm_ps[0:1, 0:cols],
                                 lhsT=W2_sb[:, 0:1],
                                 rhs=W2_sb[:, 0:cols],
                                 start=True, stop=True)

        keep_warm(40)

        # Software-pipelined steady state with a 4-deep skew so that every
        # op is data-ready when its engine reaches it (strict-FIFO queues):
        #   PE:  scores_i | seg_{i-2} | transpose_{i-4}
        #   ACT: exp_i | stage-copy_{i-4}
        #   DVE: mask_{i-1} | rcp_{i-3} | normalize_{i-3}
        for i in range(plan.nchunks + 4):
            if i < plan.nchunks:
                ch = plan.chunks[i]
                ns = ch["ns"]
                kvt = kvts[i]
                ksec = kvt[:, 0:CHUNK]
                Msec = kvt[:, SOFF + NT * ns: SOFF + (NT + 4) * ns]
                score_ps = pp.tile([TILE, NT * TILE], f32, tag="score")
                for t in range(NT):
                    nc.tensor.matmul(
                        out=score_ps[:, t * TILE: t * TILE + 4 * ns],
                        lhsT=ksec[:, t * TILE:(t + 1) * TILE],
                        rhs=Msec[:],
                        start=True, stop=True)
                st[i] = {"ns": ns, "score": score_ps, "ch": ch}

            if 0 <= i - 2 < plan.nchunks:
                keep_warm(4, 64)
                s = st[i - 2]
                ns = s["ns"]
                kvt = kvts[i - 2]
                vsec = kvt[:, VOFF:SOFF]
                park = pp.tile([TILE, 129], f32, tag="park")
                for t in range(NT):
                    nc.tensor.matmul(
                        out=park[0:4 * ns, :],
                        lhsT=s["exm"][:, t * TILE: t * TILE + 4 * ns],
                        rhs=vsec[:, 129 * t: 129 * t + 129],
                        start=(t == 0), stop=(t == NT - 1))
                s["park"] = park

            if 0 <= i - 4 < plan.nchunks:
                s = st[i - 4]
                ns = s["ns"]
                tp_ps = pp1.tile([TILE, TILE], f16, tag="tp")
                nc.tensor.transpose(out=tp_ps[0:TILE, 0:4 * ns],
                                    in_=s["aggN"][0:4 * ns, :],
                                    identity=ID_sb[0:4 * ns, 0:4 * ns])
                s["tp"] = tp_ps
                keep_warm(4, 64)

            if i < plan.nchunks:
                s = st[i]
                ns = s["ns"]
                exs = xp.tile([TILE, NT * TILE], f16, tag="exs")
                score_r = s["score"][:].rearrange("p (t c) -> p t c", t=NT)
                exs_r = exs[:].rearrange("p (t c) -> p t c", t=NT)
                nc.scalar.activation(out=exs_r[:, :, 0:4 * ns],
                                     in_=score_r[:, :, 0:4 * ns],
                                     func=Exp, bias=ln16_sb[:, 0:1])
                s["exs"] = exs

            if 0 <= i - 4 < plan.nchunks:
                s = st[i - 4]
                ns = s["ns"]
                g0 = s["ch"]["g0"]
                nc.scalar.copy(out=stag[:, 4 * g0: 4 * (g0 + ns)],
                               in_=s["tp"][0:TILE, 0:4 * ns])
                while (next_block < NBLK
                       and (g0 + ns) * 4 >= (next_block + 1) * 2048):
                    emit_block(next_block)
                    next_block += 1

            if 0 <= i - 1 < plan.nchunks:
                s = st[i - 1]
                ns = s["ns"]
                kvt = kvts[i - 1]
                Ssec = kvt[:, SOFF:SOFF + NT * ns]
                exm = xp.tile([TILE, NT * TILE], f16, tag="exm")
                exm_r = exm[:].rearrange("p (t c) -> p t c", t=NT)
                exs_r = s["exs"][:].rearrange("p (t c) -> p t c", t=NT)
                nc.vector.tensor_tensor(
                    out=exm_r[:, :, 0:4 * ns].rearrange(
                        "p t (j h) -> p t j h", h=4),
                    in0=exs_r[:, :, 0:4 * ns].rearrange(
                        "p t (j h) -> p t j h", h=4),
                    in1=Ssec[:].rearrange("p (t j) -> p t j", t=NT)
                        .unsqueeze(-1).to_broadcast([TILE, NT, ns, 4]),
                    op=mult)
                s["exm"] = exm

            if 0 <= i - 3 < plan.nchunks:
                s = st[i - 3]
                ns = s["ns"]
                park = s["park"]
                rdent = xp.tile([TILE, 1], f32, tag="rden")
                nc.vector.reciprocal(out=rdent[0:4 * ns, 0:1],
                                     in_=park[0:4 * ns, 128:129])
                aggN = xp.tile([TILE, TILE], f16, tag="aggN")
                nc.vector.tensor_tensor(
                    out=aggN[0:4 * ns, :],
                    in0=park[0:4 * ns, 0:128],
                    in1=rdent[0:4 * ns, 0:1].to_broadcast([4 * ns, TILE]),
                    op=mult)
                s["aggN"] = aggN

            if i >= 4:
                st.pop(i - 4, None)
            dma_chunk(i + PF)

        while next_block < NBLK:
            emit_block(next_block)
            next_block += 1

    nc.compile()
    return nc


# ---------------------------------------------------------------------------
# Entry point
# ---------------------------------------------------------------------------

def _prepare(inputs):
    q_nodes = np.asarray(inputs["q_nodes"], np.float32)
    k_edges = np.asarray(inputs["k_edges"], np.float32)
    v_edges = np.asarray(inputs["v_edges"], np.float32)
    Wq = np.asarray(inputs["Wq"], np.float32)
    bq = np.asarray(inputs["bq"], np.float32)
    Wk = np.asarray(inputs["Wk"], np.float32)
    Wv = np.asarray(inputs["Wv"], np.float32)
    bv = np.asarray(inputs["bv"], np.float32)
    Wo = np.asarray(inputs["Wo"], np.float32)
    bo = np.asarray(inputs["bo"], np.float32)
    dst = np.asarray(inputs["edge_index"])[0].astype(np.int64)

    plan = _make_plan(dst)

    eorder = np.argsort(dst, kind="stable")
    starts = np.zeros(N + 1, np.int64)
    np.cumsum(np.bincount(dst, minlength=N), out=starts[1:])
    edges_of = [eorder[starts[n]: starts[n + 1]] for n in range(N)]

    # host-side per-node score matrices M[d, n, h] and folded weights
    qp = q_nodes @ Wq + bq
    Mfull = np.empty((DIM, N, HEADS), np.float32)
    for h in range(HEADS):
        sl = slice(h * DH, (h + 1) * DH)
        Mfull[:, :, h] = (Wk[:, sl] * SCALE) @ qp[:, sl].T
    Mfull = Mfull.astype(np.float16)

    W2 = np.empty((DIM, 4 * DIM), np.float32)
    for h in range(HEADS):
        sl = slice(h * DH, (h + 1) * DH)
        W2[:, h * DIM:(h + 1) * DIM] = Wv[:, sl] @ Wo[sl, :]
    consts = {
        "W2": W2.astype(np.float16),
        "bo2": (bv @ Wo + bo).reshape(DIM, 1).astype(np.float32),
        "ID": np.eye(DIM, dtype=np.float16),
    }

    kT_ext = np.zeros((DIM, E + 1), np.float16)
    kT_ext[:, :E] = k_edges.T.astype(np.float16)
    v_ext = np.zeros((E + 1, DIM), np.float16)
    v_ext[:E] = v_edges.astype(np.float16)

    return plan, edges_of, consts, kT_ext, v_ext, Mfull, bo


def kernel(**inputs):
    from concourse.bass_utils import run_bass_kernel_spmd

    plan, edges_of, consts, kT_ext, v_ext, Mfull, bo = _prepare(inputs)

    nc = _build_module(plan)

    in_maps = []
    for c in range(NCORES):
        kvs = _pack_core_inputs(plan, c, kT_ext, v_ext, Mfull, edges_of)
        m = {"kvs": kvs}
        m.update(consts)
        in_maps.append(m)

    res = run_bass_kernel_spmd(nc, in_maps, core_ids=list(range(NCORES)))
    global LAST_RESULTS
    LAST_RESULTS = res

    out = np.zeros((N, DIM), np.float32)
    for c in range(NCORES):
        outT = res.results[c]["outT"]              # [128, SLOTP]
        cn = plan.core_nodes[c]
        gs = plan.rank2g[: len(cn)]
        out[np.array(cn, np.int64)] = outT[:, gs].T
    deg0 = plan.deg == 0
    if deg0.any():
        out[deg0] = bo
    return out
